# revision 31
# baseline (speedup 1.0000x reference)
"""Multi-head causal attention (B=8, S=1024, C=1024, H=16, dk=dv=64) on 8 trn2 cores.

Sharding: data-parallel over batch. Each NeuronCore processes one batch element
end-to-end (projections + attention + output projection); no collectives.

Per-core layout:
  inputs (host-prepped): xq/xk/xv = X^T [C, S] bf16, packed weights
  wq/wk [C, H*DK] (wq pre-scaled by 1/sqrt(dk)), wv [C, H*DV], wo [H*DV, C],
  biases in per-partition / replicated layouts.

  QT = wq.T @ xq  -> [H*DK, S]   (head-major rows)
  KT = wk.T @ xk  -> [H*DK, S]   (zero-padded per-head tiles for K=128 St)
  V  = xv.T @ wv  -> [S, H*DV]   (+ appended ones column per head)

  Attention runs in exact-causal q-chunks of 256: for q-chunk j only key
  blocks 0..2j+1 are computed (the last block only for its live 128 q-cols).
  Score PSUM for up to 4 key blocks is merged so one ACT Exp instruction
  covers ~1K columns; the causal diagonal is a post-exp triangular multiply
  on DVE. P@[V|1] accumulates O^T rows plus the softmax denominator row; the
  denominator is replicated across partitions with a tiny K=2 matmul and
  inverted with the fast DVE reciprocal, keeping the ACT engine exclusively
  on Exp. Output projection per finished 128-q-row tile.
"""

import math
import os
import sys

import numpy as np

try:
    import concourse.bass as bass
except ImportError:  # make concourse importable in a bare grading dir
    for _p in ("/opt/trn_rl_repo", os.path.expanduser("~/.axon_site/_ro/trn_rl_repo")):
        if os.path.isdir(_p) and _p not in sys.path:
            sys.path.insert(0, _p)
    import concourse.bass as bass

from contextlib import ExitStack

import ml_dtypes

import concourse.mybir as mybir
import concourse.tile as tile
from concourse import bacc
from concourse.bass_utils import run_bass_kernel_spmd

def _setup_act_tables():
    """Pin the ACT function table to the set that covers exp+ln+identity+copy
    so the kernel never reloads LUTs mid-flight. Both the bacc-side pass and
    walrus must see the same (reordered) act_info.json."""
    import json
    import shutil
    import tempfile

    import concourse.hw_specs as hw_specs
    from concourse import bacc as _bacc

    if os.environ.get("BASS_ACT_ROOT_JSON_PATH"):
        return  # already configured
    from neuronxcc.driver.Job import Job

    orig = os.path.join(
        Job.getPackageDir(), "pwp", "pwp_bin_trainium", "act_info.json"
    )
    assert os.path.isfile(orig), orig
    dst = os.path.join(tempfile.gettempdir(), "mha_act_tables")
    if not os.path.isdir(dst):
        tmp = dst + ".tmp"
        shutil.rmtree(tmp, ignore_errors=True)
        shutil.copytree(os.path.dirname(orig), tmp)
        with open(os.path.join(tmp, "act_info.json")) as f:
            info = json.load(f)
        sets = info["act_func_sets"]
        want = [s for s in sets if s["name"] == "natural_log_exp_and_others"]
        rest = [s for s in sets if s["name"] != "natural_log_exp_and_others"]
        info["act_func_sets"] = want + rest
        with open(os.path.join(tmp, "act_info.json"), "w") as f:
            json.dump(info, f)
        os.replace(tmp, dst)
    path = os.path.join(dst, "act_info.json")
    os.environ["BASS_ACT_ROOT_JSON_PATH"] = path

    def patched(module_arch):
        with open(path) as af:
            act_info = json.load(af)
        return {
            ent["name"]: {
                mybir.ActivationFunctionType.from_pwp(v) for v in ent["act"].keys()
            }
            for ent in act_info["act_func_sets"]
        }

    hw_specs.get_activation_tables = patched
    _bacc.get_activation_tables = patched
    from concourse import bass_interp as _bi

    _bi.get_activation_tables = patched


B, S, C = 8, 1024, 1024
H, DK, DV = 16, 64, 64
P = 128
NT = 8  # number of 128-tiles along S / C / H*DK
CH = 512  # projection free-dim chunk (one PSUM bank of fp32)
NCH = S // CH
QC = 512  # attention q-chunk
NQC = S // QC


FP = mybir.dt.float32
BF = mybir.dt.bfloat16
BF_NP = ml_dtypes.bfloat16
AFT = mybir.ActivationFunctionType
ALU = mybir.AluOpType


def build_nc() -> bass.Bass:
    _setup_act_tables()
    nc = bacc.Bacc()

    xq = nc.dram_tensor("xq", [C, S], BF, kind="ExternalInput")
    xk = nc.dram_tensor("xk", [C, S], BF, kind="ExternalInput")
    xv = nc.dram_tensor("xv", [C, S], BF, kind="ExternalInput")
    wq = nc.dram_tensor("wq", [C, H * DK], BF, kind="ExternalInput")
    wk = nc.dram_tensor("wk", [C, H * DK], BF, kind="ExternalInput")
    wv = nc.dram_tensor("wv", [C, H * DV], BF, kind="ExternalInput")
    wo = nc.dram_tensor("wo", [H * DV, C], BF, kind="ExternalInput")
    bqd = nc.dram_tensor("bq", [P, NT], FP, kind="ExternalInput")
    bkd = nc.dram_tensor("bk", [P, NT], FP, kind="ExternalInput")
    bvd = nc.dram_tensor("bv", [P, H * DV], BF, kind="ExternalInput")
    bod = nc.dram_tensor("bo", [1, C], BF, kind="ExternalInput")
    y = nc.dram_tensor("y", [S, C], FP, kind="ExternalOutput")

    # constants baked into the NEFF, one bf16 tensor (single DMA):
    # cols 0:640   = causal mask for a band-block pair stored as [512|384]:
    #                [tri | ones(384) | tri]  (tri[t,q] = 1 iff t<=q)
    # cols 640:1024 = mask for a pair stored as [256|128]: [tri | ones | tri]
    tri = np.triu(np.ones((P, P), np.float32))
    ones = np.ones((P, P), np.float32)
    pat2 = np.zeros((P, P), np.float32)
    pat2[0, 0:DV] = 1.0
    pat2[32, DV:P] = 1.0
    cpack_np = np.concatenate(
        [tri, ones, ones, ones, tri, tri, ones, tri, pat2], axis=1
    )
    cpack_d = nc.inline_tensor(cpack_np.astype(BF_NP), "cpack")

    xq_r = xq.rearrange("(ko p) s -> p ko s", p=P)
    xk_r = xk.rearrange("(ko p) s -> p ko s", p=P)
    xv_r = xv.rearrange("(ko p) s -> p ko s", p=P)
    wq_r = wq.rearrange("(ko p) m -> p ko m", p=P)
    wk_r = wk.rearrange("(ko p) m -> p ko m", p=P)
    wv_r = wv.rearrange("(ko p) m -> p ko m", p=P)
    wo_r = wo.rearrange("(ko p) c -> p ko c", p=P)
    y_r = y.rearrange("(mo p) c -> p mo c", p=P)

    with tile.TileContext(nc) as tc, ExitStack() as octx:
        const = octx.enter_context(tc.tile_pool(name="const", bufs=1))
        qk = octx.enter_context(tc.tile_pool(name="qk", bufs=1))
        opool = octx.enter_context(tc.tile_pool(name="oT", bufs=1))

        qT_sb = qk.tile([P, NT, S], BF, tag="qT")
        kT2_sb = qk.tile([P, H, S], BF, tag="kT2")
        v_sb = qk.tile([P, NT, H, DV + 1], BF, tag="v")
        oT_sb = opool.tile([P, NT, S], BF, tag="oT")

        # ---------------- pools + DMAs ----------------
        wpool = octx.enter_context(tc.tile_pool(name="wqkv", bufs=2))
        xpool = octx.enter_context(tc.tile_pool(name="xin", bufs=2))

        wq_sb = wpool.tile([P, NT, H * DK], BF, tag="w", name="wq_sb")
        xq_sb = xpool.tile([P, NT, S], BF, tag="x", name="xq_sb")
        wk_sb = wpool.tile([P, NT, H * DK], BF, tag="w", name="wk_sb")
        xk_sb = xpool.tile([P, NT, S], BF, tag="x", name="xk_sb")
        # Q inputs + small consts first so the PE starts ASAP
        nc.sync.dma_start(wq_sb[:, 0], wq_r[:, 0])
        nc.sync.dma_start(xq_sb[:, 0], xq_r[:, 0])
        bq_sb = const.tile([P, NT], FP, tag="bq")
        nc.sync.dma_start(bq_sb, bqd[:])
        bk_sb = const.tile([P, NT], FP, tag="bk")
        nc.sync.dma_start(bk_sb, bkd[:])
        cpack_sb = const.tile([P, 1152], BF, tag="cpack")
        nc.sync.dma_start(cpack_sb, cpack_d[:])
        for kc in range(1, NT):
            nc.sync.dma_start(wq_sb[:, kc], wq_r[:, kc])
            nc.sync.dma_start(xq_sb[:, kc], xq_r[:, kc])
        for kc in range(NT):
            nc.sync.dma_start(wk_sb[:, kc], wk_r[:, kc])
            nc.sync.dma_start(xk_sb[:, kc], xk_r[:, kc])

        bo_sb = const.tile([1, C], BF, tag="bo")
        nc.sync.dma_start(bo_sb, bod[:])
        borep_sb = const.tile([P, C], BF, tag="borep")
        nc.gpsimd.partition_broadcast(borep_sb, bo_sb)
        bv_sb = const.tile([P, H * DV], BF, tag="bv")
        nc.sync.dma_start(bv_sb, bvd[:])

        nc.vector.memset(v_sb[:, :, :, DV], 1.0)
        # zero the unused half of each head's K^T tile so St matmuls can
        # contract over the full 128 partitions (the zero lhsT rows
        # nullify the other head's Q rows)
        for h in range(H):
            hz = DK if h % 2 == 0 else 0
            nc.gpsimd.memset(kT2_sb[hz : hz + DK, h, :], 0.0)

        # V inputs reuse Q's buffers (free once Q's matmuls finish); the
        # output-projection weight reuses K's buffer after K's last matmul
        wv_sb = wpool.tile([P, NT, H * DV], BF, tag="w", name="wv_sb")
        xv_sb = xpool.tile([P, NT, S], BF, tag="x", name="xv_sb")
        for kc in range(NT):
            nc.sync.dma_start(wv_sb[:, kc], wv_r[:, kc])
            nc.sync.dma_start(xv_sb[:, kc], xv_r[:, kc])
        wo_sb = wpool.tile([P, NT, C], BF, tag="w", name="wo_sb")
        nc.sync.dma_start(wo_sb, wo_r)

        # ---------------- Q + K(n=0) projections ----------------
        # out[hk, s]; lhsT = w tile [c, hk], rhs = x^T [c, s]; kc-outer so
        # matmuls chase the input DMAs; evacuation (+bias) on the idle ACT.
        def qk_evac(psums, n, b_sb, out_sb):
            sl = slice(n * CH, (n + 1) * CH)
            for m in range(NT):
                if out_sb is not None:
                    nc.scalar.activation(
                        out_sb[:, m, sl], psums[m], AFT.Identity,
                        bias=b_sb[:, m : m + 1],
                    )
                else:  # K^T: split the head pair into per-head tiles
                    nc.scalar.activation(
                        kT2_sb[0:DK, 2 * m, sl], psums[m][0:DK],
                        AFT.Identity, bias=b_sb[0:DK, m : m + 1],
                    )
                    nc.scalar.activation(
                        kT2_sb[DK:P, 2 * m + 1, sl], psums[m][DK:P],
                        AFT.Identity, bias=b_sb[DK:P, m : m + 1],
                    )

        with ExitStack() as ictx:
            psproj = ictx.enter_context(
                tc.tile_pool(name="psproj", bufs=8, space="PSUM")
            )
            for w_sb, x_sb, b_sb, out_sb, n in (
                (wq_sb, xq_sb, bq_sb, qT_sb, 0),
                (wq_sb, xq_sb, bq_sb, qT_sb, 1),
                (wk_sb, xk_sb, bk_sb, None, 0),
            ):
                psums = []
                for kc in range(NT):
                    for m in range(NT):
                        if kc == 0:
                            psums.append(psproj.tile(
                                [P, CH], FP, tag="proj", name=f"proj_ps_{m}"))
                        nc.tensor.matmul(
                            psums[m],
                            w_sb[:, kc, m * P : (m + 1) * P],
                            x_sb[:, kc, n * CH : (n + 1) * CH],
                            start=(kc == 0), stop=(kc == NT - 1),
                        )
                qk_evac(psums, n, b_sb, out_sb)

        # remaining PSUM pools: 3 (mix) + 3 (scores) + 2 (P@V out) = 8 banks
        ps_mix = octx.enter_context(tc.tile_pool(name="ps_mix", bufs=2, space="PSUM"))
        ps_st = octx.enter_context(tc.tile_pool(name="ps_st", bufs=2, space="PSUM"))
        ps_o = octx.enter_context(tc.tile_pool(name="ps_o", bufs=2, space="PSUM"))
        ppool = octx.enter_context(tc.tile_pool(name="p", bufs=16))
        rpool = octx.enter_context(tc.tile_pool(name="r", bufs=3))
        oupool = octx.enter_context(tc.tile_pool(name="ou", bufs=4))
        ypool = octx.enter_context(tc.tile_pool(name="y", bufs=2))

        def kn1_piece(m):
            """K projection n=1 for one m-tile (filler between ch0 stages)."""
            ps = ps_mix.tile([P, CH], FP, tag="mix", name=f"kn1_{m}")
            for kc in range(NT):
                nc.tensor.matmul(
                    ps, wk_sb[:, kc, m * P : (m + 1) * P],
                    xk_sb[:, kc, CH : 2 * CH],
                    start=(kc == 0), stop=(kc == NT - 1),
                )
            sl = slice(CH, 2 * CH)
            nc.scalar.activation(
                kT2_sb[0:DK, 2 * m, sl], ps[0:DK], AFT.Identity,
                bias=bk_sb[0:DK, m : m + 1],
            )
            nc.scalar.activation(
                kT2_sb[DK:P, 2 * m + 1, sl], ps[DK:P], AFT.Identity,
                bias=bk_sb[DK:P, m : m + 1],
            )

        def vm_piece(m):
            """V projection for key block m: out[s, hv]; lhsT = x^T tile."""
            pss = []
            for nh in range(NCH):
                ps = ps_mix.tile([P, CH], FP, tag="mix", name=f"v_{m}_{nh}")
                pss.append(ps)
            for kc in range(NT):
                for nh in range(NCH):
                    nc.tensor.matmul(
                        pss[nh], xv_sb[:, kc, m * P : (m + 1) * P],
                        wv_sb[:, kc, nh * CH : (nh + 1) * CH],
                        start=(kc == 0), stop=(kc == NT - 1),
                    )
            for nh in range(NCH):
                nc.vector.tensor_tensor(
                    v_sb[:, m, 8 * nh : 8 * (nh + 1), 0:DV],
                    pss[nh].rearrange("p (h v) -> p h v", v=DV),
                    bv_sb[:, nh * CH : (nh + 1) * CH].rearrange(
                        "p (h v) -> p h v", v=DV
                    ),
                    ALU.add,
                )

        # ---------------- attention + interleaved projections ------------
        # Chunk 0 (keys 0:512) needs only K n=0 and V blocks 0..3, so its
        # ACT-bound scores+exp interleave with the K n=1 matmuls as PE
        # filler; chunk 1 software-pipelines with chunk-0 output-projection
        # pieces. Filler is woven BETWEEN score groups so the in-order PE
        # never stalls on the score-PSUM ring while work waits behind it.
        state = {}

        def blocks_of(j):
            """(i, qoff, width, group, loc): key blocks for q-chunk j packed
            pairwise into score-PSUM groups. The last 4 blocks form the
            causal diagonal band; block 4j+k only covers q-cols 128k:512."""
            out = []
            for i in range(4 * j + 4):
                qoff = max(0, (i - 4 * j) * 128)
                g, first = divmod(i, 2)
                loc = 0 if first == 0 else out[-1][2]
                out.append((i, qoff, QC - qoff, g, loc))
            return out

        def st_sub(j, hp, sub, g):
            """Scores + exp + causal mask for one pair-group of one head."""
            qlo = j * QC
            st = state.setdefault((j, hp), {"pchs": {}})
            gb = [b for b in blocks_of(j) if b[3] == g]
            h = 2 * hp + sub
            stp = ps_st.tile([P, 1024], FP, tag="st", name=f"st_{j}_{hp}_{sub}_{g}")
            for i, qoff, w, _, loc in gb:
                nc.tensor.matmul(
                    stp[:, loc : loc + w],
                    kT2_sb[:, h, i * P : (i + 1) * P],
                    qT_sb[:, hp, qlo + qoff : qlo + QC],
                    start=True,
                    stop=True,
                )
            span = gb[-1][4] + gb[-1][2]
            pch = ppool.tile([P, 1024], BF, tag="p", name=f"p_{j}_{hp}_{sub}_{g}")
            nc.scalar.activation(pch[:, 0:span], stp[:, 0:span], AFT.Exp)
            if gb[0][0] >= 4 * j:  # band pair: combined triangular mask
                if gb[0][2] == QC:  # [512|384] pair
                    nc.vector.tensor_tensor(
                        pch[:, 0:640], pch[:, 0:640], cpack_sb[:, 0:640], ALU.mult
                    )
                else:  # [256|128] pair
                    nc.vector.tensor_tensor(
                        pch[:, 0:384], pch[:, 0:384], cpack_sb[:, 640:1024], ALU.mult
                    )
            st["pchs"][(sub, g)] = pch

        def pv_sub(j, hp, sub):
            st = state[(j, hp)]

            h = 2 * hp + sub
            blks = blocks_of(j)
            pos = ps_o.tile([P, QC], FP, tag="o", name=f"po_{j}_{hp}_{sub}")
            for i, qoff, w, g, loc in blks:
                nc.tensor.matmul(
                    pos[0 : DV + 1, qoff:QC],
                    v_sb[:, i, h, :],
                    st["pchs"][(sub, g)][:, loc : loc + w],
                    start=(i == 0),
                    stop=(i == len(blks) - 1),
                )
            # immediate PSUM->SBUF evacuation so the bank recycles fast
            ou = oupool.tile([DV, QC], FP, tag="ou", name=f"ou_{j}_{hp}_{sub}")
            nc.vector.tensor_copy(out=ou, in_=pos[0:DV])
            r1 = rpool.tile([1, QC], BF, tag="r1", name=f"r1_{j}_{hp}_{sub}")
            nc.vector.tensor_copy(out=r1, in_=pos[DV : DV + 1])
            st[("r1", sub)] = r1
            st[("ou", sub)] = ou

        def norm_stage(j, hp):
            """oT = ou / r: one K=2 matmul replicates both subs' denominator
            rows across the partition halves; fast reciprocal + mults on DVE."""
            st = state.pop((j, hp))
            qlo = j * QC
            for sub in (0, 1):
                hm = sub * DV
                rrep = ps_mix.tile(
                    [DV, CH], FP, tag="mix", name=f"rrep_{j}_{hp}_{sub}"
                )
                nc.tensor.matmul(
                    rrep[:, 0:QC], cpack_sb[0:1, P : P + DV], st[("r1", sub)],
                    start=True, stop=True,
                )
                rrinv = rpool.tile(
                    [DV, QC], FP, tag="rrinv", name=f"rrinv_{j}_{hp}_{sub}"
                )
                nc.vector.reciprocal_approx_fast(rrinv, rrep[:, 0:QC])
                nc.vector.tensor_tensor(
                    oT_sb[hm : hm + DV, hp, qlo : qlo + QC],
                    st[("ou", sub)],
                    rrinv,
                    ALU.mult,
                )

        def outproj_piece(m, n):
            py = ps_mix.tile([P, CH], FP, tag="mix", name=f"py_{m}_{n}")
            for kc in range(NT):
                nc.tensor.matmul(
                    py,
                    oT_sb[:, kc, m * P : (m + 1) * P],
                    wo_sb[:, kc, n * CH : (n + 1) * CH],
                    start=(kc == 0),
                    stop=(kc == NT - 1),
                )
            yt = ypool.tile([P, CH], FP, tag="y")
            nc.vector.tensor_tensor(
                yt, py, borep_sb[:, n * CH : (n + 1) * CH], ALU.add
            )
            nc.sync.dma_start(y_r[:, m, n * CH : (n + 1) * CH], yt)

        # V blocks 0..3 (overlaps the tail of the Q/K evacuations on ACT)
        for m in range(4):
            vm_piece(m)

        # chunk 0, K n=1 pieces woven between score groups
        kn1_q = list(range(NT))
        for hp in range(H // 2):
            st_sub(0, hp, 0, 0)
            st_sub(0, hp, 0, 1)
            if kn1_q:
                kn1_piece(kn1_q.pop(0))
            if hp >= 2:
                pv_sub(0, hp - 2, 0)
            st_sub(0, hp, 1, 0)
            st_sub(0, hp, 1, 1)
            if kn1_q:
                kn1_piece(kn1_q.pop(0))
            if hp >= 2:
                pv_sub(0, hp - 2, 1)
            if hp >= 3:
                norm_stage(0, hp - 3)
        # drain chunk 0, V blocks 4..7 interleaved
        for sub in (0, 1):
            pv_sub(0, 6, sub)
        vm_piece(4)
        norm_stage(0, 5)
        vm_piece(5)
        for sub in (0, 1):
            pv_sub(0, 7, sub)
        vm_piece(6)
        norm_stage(0, 6)
        vm_piece(7)
        norm_stage(0, 7)

        op_queue = [(m, n) for m in range(4) for n in range(NCH)]

        # chunk 1 software-pipelined with chunk-0 output-projection filler
        for hp in range(H // 2):
            st_sub(1, hp, 0, 0)
            st_sub(1, hp, 0, 1)
            if hp >= 1:
                pv_sub(1, hp - 1, 0)
            st_sub(1, hp, 0, 2)
            st_sub(1, hp, 0, 3)
            if hp >= 1:
                pv_sub(1, hp - 1, 1)
            st_sub(1, hp, 1, 0)
            st_sub(1, hp, 1, 1)
            outproj_piece(*op_queue.pop(0))
            st_sub(1, hp, 1, 2)
            st_sub(1, hp, 1, 3)
            if hp >= 2:
                norm_stage(1, hp - 2)
        for sub in (0, 1):
            pv_sub(1, 7, sub)
        norm_stage(1, 6)
        norm_stage(1, 7)
        op_queue += [(m, n) for m in range(4, NT) for n in range(NCH)]
        while op_queue:
            outproj_piece(*op_queue.pop(0))

    nc.finalize()
    return nc


_NC_CACHE = None


def _get_nc() -> bass.Bass:
    global _NC_CACHE
    if _NC_CACHE is None:
        _NC_CACHE = build_nc()
    return _NC_CACHE


def prep_shared(Wq, bq, Wk, bk, Wv, bv, Wo, bo):
    """Host-side packing of weights/biases (shared by all cores)."""
    scale = 1.0 / math.sqrt(DK)
    Wq = np.asarray(Wq, np.float32)
    Wk = np.asarray(Wk, np.float32)
    Wv = np.asarray(Wv, np.float32)
    Wo = np.asarray(Wo, np.float32)
    out = {
        "wq": np.ascontiguousarray(
            (Wq.transpose(1, 0, 2).reshape(C, H * DK) * scale).astype(BF_NP)
        ),
        "wk": np.ascontiguousarray(
            Wk.transpose(1, 0, 2).reshape(C, H * DK).astype(BF_NP)
        ),
        "wv": np.ascontiguousarray(
            Wv.transpose(1, 0, 2).reshape(C, H * DV).astype(BF_NP)
        ),
        "wo": Wo.astype(BF_NP),
        "bq": np.ascontiguousarray(
            (np.asarray(bq, np.float32).reshape(H * DK) * scale)
            .reshape(NT, P)
            .T.astype(np.float32)
        ),
        "bk": np.ascontiguousarray(
            np.asarray(bk, np.float32).reshape(NT, P).T.astype(np.float32)
        ),
        "bv": np.ascontiguousarray(
            np.broadcast_to(
                np.asarray(bv, np.float32).reshape(1, H * DV), (P, H * DV)
            ).astype(BF_NP)
        ),
        "bo": np.ascontiguousarray(
            np.asarray(bo, np.float32).reshape(1, C).astype(BF_NP)
        ),
    }
    return out


def prep_core(q_embs_b, k_embs_b, v_embs_b):
    return {
        "xq": np.ascontiguousarray(np.asarray(q_embs_b, np.float32).T.astype(BF_NP)),
        "xk": np.ascontiguousarray(np.asarray(k_embs_b, np.float32).T.astype(BF_NP)),
        "xv": np.ascontiguousarray(np.asarray(v_embs_b, np.float32).T.astype(BF_NP)),
    }


def kernel(q_embs, k_embs, v_embs, Wq, bq, Wk, bk, Wv, bv, Wo, bo, **run_kwargs):
    nc = _get_nc()
    shared = prep_shared(Wq, bq, Wk, bk, Wv, bv, Wo, bo)
    q_embs = np.asarray(q_embs, np.float32)
    k_embs = np.asarray(k_embs, np.float32)
    v_embs = np.asarray(v_embs, np.float32)
    in_maps = []
    for b in range(B):
        m = dict(shared)
        m.update(prep_core(q_embs[b], k_embs[b], v_embs[b]))
        in_maps.append(m)
    res = run_bass_kernel_spmd(nc, in_maps, core_ids=list(range(B)), **run_kwargs)
    out = np.stack([res.results[i]["y"] for i in range(B)], axis=0)
    if run_kwargs:
        kernel.last_results = res
    return out


if __name__ == "__main__":
    rng = np.random.default_rng(0)
    inputs = {
        "q_embs": rng.standard_normal((B, S, C), np.float32),
        "k_embs": rng.standard_normal((B, S, C), np.float32),
        "v_embs": rng.standard_normal((B, S, C), np.float32),
        "Wq": rng.standard_normal((H, C, DK), np.float32) * 0.02,
        "bq": np.zeros((H, DK), np.float32),
        "Wk": rng.standard_normal((H, C, DK), np.float32) * 0.02,
        "bk": np.zeros((H, DK), np.float32),
        "Wv": rng.standard_normal((H, C, DV), np.float32) * 0.02,
        "bv": np.zeros((H, DV), np.float32),
        "Wo": rng.standard_normal((H * DV, C), np.float32) * 0.02,
        "bo": np.zeros((C,), np.float32),
    }
    out = kernel(**inputs)
    print(out.shape, out.dtype)


# revision 33
# speedup vs baseline: 1.2058x; 1.2058x over previous
"""Multi-head causal attention (B=8, S=1024, C=1024, H=16, dk=dv=64) on 8 trn2 cores.

Sharding: data-parallel over batch. Each NeuronCore processes one batch element
end-to-end (projections + attention + output projection); no collectives.

Per-core layout:
  inputs (host-prepped): xq/xk/xv = X^T [C, S] bf16, packed weights
  wq/wk [C, H*DK] (wq pre-scaled by 1/sqrt(dk)), wv [C, H*DV], wo [H*DV, C],
  biases in per-partition / replicated layouts.

  QT = wq.T @ xq  -> [H*DK, S]   (head-major rows)
  KT = wk.T @ xk  -> [H*DK, S]   (zero-padded per-head tiles for K=128 St)
  V  = xv.T @ wv  -> [S, H*DV]   (+ appended ones column per head)

  Attention runs in exact-causal q-chunks of 256: for q-chunk j only key
  blocks 0..2j+1 are computed (the last block only for its live 128 q-cols).
  Score PSUM for up to 4 key blocks is merged so one ACT Exp instruction
  covers ~1K columns; the causal diagonal is a post-exp triangular multiply
  on DVE. P@[V|1] accumulates O^T rows plus the softmax denominator row; the
  denominator is replicated across partitions with a tiny K=2 matmul and
  inverted with the fast DVE reciprocal, keeping the ACT engine exclusively
  on Exp. Output projection per finished 128-q-row tile.
"""

import math
import os
import sys

import numpy as np

try:
    import concourse.bass as bass
except ImportError:  # make concourse importable in a bare grading dir
    for _p in ("/opt/trn_rl_repo", os.path.expanduser("~/.axon_site/_ro/trn_rl_repo")):
        if os.path.isdir(_p) and _p not in sys.path:
            sys.path.insert(0, _p)
    import concourse.bass as bass

from contextlib import ExitStack

import ml_dtypes

import concourse.mybir as mybir
import concourse.tile as tile
from concourse import bacc
from concourse.bass_utils import run_bass_kernel_spmd

def _setup_act_tables():
    """Pin the ACT function table to the set that covers exp+ln+identity+copy
    so the kernel never reloads LUTs mid-flight. Both the bacc-side pass and
    walrus must see the same (reordered) act_info.json."""
    import json
    import shutil
    import tempfile

    import concourse.hw_specs as hw_specs
    from concourse import bacc as _bacc

    if os.environ.get("BASS_ACT_ROOT_JSON_PATH"):
        return  # already configured
    from neuronxcc.driver.Job import Job

    orig = os.path.join(
        Job.getPackageDir(), "pwp", "pwp_bin_trainium", "act_info.json"
    )
    assert os.path.isfile(orig), orig
    dst = os.path.join(tempfile.gettempdir(), "mha_act_tables")
    if not os.path.isdir(dst):
        tmp = dst + ".tmp"
        shutil.rmtree(tmp, ignore_errors=True)
        shutil.copytree(os.path.dirname(orig), tmp)
        with open(os.path.join(tmp, "act_info.json")) as f:
            info = json.load(f)
        sets = info["act_func_sets"]
        want = [s for s in sets if s["name"] == "natural_log_exp_and_others"]
        rest = [s for s in sets if s["name"] != "natural_log_exp_and_others"]
        info["act_func_sets"] = want + rest
        with open(os.path.join(tmp, "act_info.json"), "w") as f:
            json.dump(info, f)
        os.replace(tmp, dst)
    path = os.path.join(dst, "act_info.json")
    os.environ["BASS_ACT_ROOT_JSON_PATH"] = path

    def patched(module_arch):
        with open(path) as af:
            act_info = json.load(af)
        return {
            ent["name"]: {
                mybir.ActivationFunctionType.from_pwp(v) for v in ent["act"].keys()
            }
            for ent in act_info["act_func_sets"]
        }

    hw_specs.get_activation_tables = patched
    _bacc.get_activation_tables = patched
    from concourse import bass_interp as _bi

    _bi.get_activation_tables = patched


B, S, C = 8, 1024, 1024
H, DK, DV = 16, 64, 64
P = 128
NT = 8  # number of 128-tiles along S / C / H*DK
CH = 512  # projection free-dim chunk (one PSUM bank of fp32)
NCH = S // CH
QC = 512  # attention q-chunk
NQC = S // QC


FP = mybir.dt.float32
BF = mybir.dt.bfloat16
BF_NP = ml_dtypes.bfloat16
AFT = mybir.ActivationFunctionType
ALU = mybir.AluOpType


def build_nc() -> bass.Bass:
    _setup_act_tables()
    nc = bacc.Bacc()

    xq = nc.dram_tensor("xq", [C, S], BF, kind="ExternalInput")
    xk = nc.dram_tensor("xk", [C, S], BF, kind="ExternalInput")
    xv = nc.dram_tensor("xv", [C, S], BF, kind="ExternalInput")
    wq = nc.dram_tensor("wq", [C, H * DK], BF, kind="ExternalInput")
    wk = nc.dram_tensor("wk", [C, H * DK], BF, kind="ExternalInput")
    wv = nc.dram_tensor("wv", [C, H * DV], BF, kind="ExternalInput")
    wo = nc.dram_tensor("wo", [H * DV, C], BF, kind="ExternalInput")
    bqd = nc.dram_tensor("bq", [P, NT], FP, kind="ExternalInput")
    bkd = nc.dram_tensor("bk", [P, NT], FP, kind="ExternalInput")
    bvd = nc.dram_tensor("bv", [P, H * DV], BF, kind="ExternalInput")
    bod = nc.dram_tensor("bo", [1, C], BF, kind="ExternalInput")
    y = nc.dram_tensor("y", [S, C], FP, kind="ExternalOutput")

    # constants baked into the NEFF, one bf16 tensor (single DMA):
    # cols 0:640   = causal mask for a band-block pair stored as [512|384]:
    #                [tri | ones(384) | tri]  (tri[t,q] = 1 iff t<=q)
    # cols 640:1024 = mask for a pair stored as [256|128]: [tri | ones | tri]
    tri = np.triu(np.ones((P, P), np.float32))
    ones = np.ones((P, P), np.float32)
    pat2 = np.zeros((P, P), np.float32)
    pat2[0, 0:DV] = 1.0
    pat2[32, DV:P] = 1.0
    cpack_np = np.concatenate(
        [tri, ones, ones, ones, tri, tri, ones, tri, pat2], axis=1
    )
    cpack_d = nc.inline_tensor(cpack_np.astype(BF_NP), "cpack")

    xq_r = xq.rearrange("(ko p) s -> p ko s", p=P)
    xk_r = xk.rearrange("(ko p) s -> p ko s", p=P)
    xv_r = xv.rearrange("(ko p) s -> p ko s", p=P)
    wq_r = wq.rearrange("(ko p) m -> p ko m", p=P)
    wk_r = wk.rearrange("(ko p) m -> p ko m", p=P)
    wv_r = wv.rearrange("(ko p) m -> p ko m", p=P)
    wo_r = wo.rearrange("(ko p) c -> p ko c", p=P)
    y_r = y.rearrange("(mo p) c -> p mo c", p=P)

    with tile.TileContext(nc) as tc, ExitStack() as octx:
        const = octx.enter_context(tc.tile_pool(name="const", bufs=1))
        qk = octx.enter_context(tc.tile_pool(name="qk", bufs=1))
        opool = octx.enter_context(tc.tile_pool(name="oT", bufs=1))

        qT_sb = qk.tile([P, NT, S], BF, tag="qT")
        kT2_sb = qk.tile([P, H, S], BF, tag="kT2")
        v_sb = qk.tile([P, NT, H, DV + 1], BF, tag="v")
        oT_sb = opool.tile([P, NT, S], BF, tag="oT")

        # ---------------- pools + DMAs ----------------
        wpool = octx.enter_context(tc.tile_pool(name="wqkv", bufs=2))
        xpool = octx.enter_context(tc.tile_pool(name="xin", bufs=2))

        wq_sb = wpool.tile([P, NT, H * DK], BF, tag="w", name="wq_sb")
        xq_sb = xpool.tile([P, NT, S], BF, tag="x", name="xq_sb")
        wk_sb = wpool.tile([P, NT, H * DK], BF, tag="w", name="wk_sb")
        xk_sb = xpool.tile([P, NT, S], BF, tag="x", name="xk_sb")
        # Q inputs + small consts first so the PE starts ASAP
        nc.sync.dma_start(wq_sb[:, 0], wq_r[:, 0])
        nc.sync.dma_start(xq_sb[:, 0], xq_r[:, 0])
        bq_sb = const.tile([P, NT], FP, tag="bq")
        nc.sync.dma_start(bq_sb, bqd[:])
        bk_sb = const.tile([P, NT], FP, tag="bk")
        nc.sync.dma_start(bk_sb, bkd[:])
        cpack_sb = const.tile([P, 1152], BF, tag="cpack")
        nc.sync.dma_start(cpack_sb, cpack_d[:])
        for kc in range(1, NT):
            nc.sync.dma_start(wq_sb[:, kc], wq_r[:, kc])
            nc.sync.dma_start(xq_sb[:, kc], xq_r[:, kc])
        for kc in range(NT):
            nc.sync.dma_start(wk_sb[:, kc], wk_r[:, kc])
            nc.sync.dma_start(xk_sb[:, kc], xk_r[:, kc])

        bo_sb = const.tile([1, C], BF, tag="bo")
        nc.sync.dma_start(bo_sb, bod[:])
        borep_sb = const.tile([P, C], BF, tag="borep")
        nc.gpsimd.partition_broadcast(borep_sb, bo_sb)
        bv_sb = const.tile([P, H * DV], BF, tag="bv")
        nc.sync.dma_start(bv_sb, bvd[:])

        nc.vector.memset(v_sb[:, :, :, DV], 1.0)
        # zero the unused half of each head's K^T tile so St matmuls can
        # contract over the full 128 partitions (the zero lhsT rows
        # nullify the other head's Q rows)
        for h in range(H):
            hz = DK if h % 2 == 0 else 0
            nc.gpsimd.memset(kT2_sb[hz : hz + DK, h, :], 0.0)

        # V inputs reuse Q's buffers (free once Q's matmuls finish); the
        # output-projection weight reuses K's buffer after K's last matmul
        wv_sb = wpool.tile([P, NT, H * DV], BF, tag="w", name="wv_sb")
        xv_sb = xpool.tile([P, NT, S], BF, tag="x", name="xv_sb")
        for kc in range(NT):
            nc.sync.dma_start(wv_sb[:, kc], wv_r[:, kc])
            nc.sync.dma_start(xv_sb[:, kc], xv_r[:, kc])
        wo_sb = wpool.tile([P, NT, C], BF, tag="w", name="wo_sb")
        nc.sync.dma_start(wo_sb, wo_r)

        # ---------------- Q + K(n=0) projections ----------------
        # out[hk, s]; lhsT = w tile [c, hk], rhs = x^T [c, s]; kc-outer so
        # matmuls chase the input DMAs; evacuation (+bias) on the idle ACT.
        def qk_evac(psums, n, b_sb, out_sb):
            sl = slice(n * CH, (n + 1) * CH)
            for m in range(NT):
                if out_sb is not None:
                    nc.vector.tensor_scalar_add(
                        out_sb[:, m, sl], psums[m], b_sb[:, m : m + 1]
                    )
                else:  # K^T: split the head pair into per-head tiles
                    nc.vector.tensor_scalar_add(
                        kT2_sb[0:DK, 2 * m, sl], psums[m][0:DK],
                        b_sb[0:DK, m : m + 1],
                    )
                    nc.vector.tensor_scalar_add(
                        kT2_sb[DK:P, 2 * m + 1, sl], psums[m][DK:P],
                        b_sb[DK:P, m : m + 1],
                    )

        with ExitStack() as ictx:
            psproj = ictx.enter_context(
                tc.tile_pool(name="psproj", bufs=8, space="PSUM")
            )
            for w_sb, x_sb, b_sb, out_sb, n in (
                (wq_sb, xq_sb, bq_sb, qT_sb, 0),
                (wq_sb, xq_sb, bq_sb, qT_sb, 1),
                (wk_sb, xk_sb, bk_sb, None, 0),
            ):
                psums = []
                for kc in range(NT):
                    for m in range(NT):
                        if kc == 0:
                            psums.append(psproj.tile(
                                [P, CH], FP, tag="proj", name=f"proj_ps_{m}"))
                        nc.tensor.matmul(
                            psums[m],
                            w_sb[:, kc, m * P : (m + 1) * P],
                            x_sb[:, kc, n * CH : (n + 1) * CH],
                            start=(kc == 0), stop=(kc == NT - 1),
                        )
                qk_evac(psums, n, b_sb, out_sb)

        # remaining PSUM pools: 3 (mix) + 3 (scores) + 2 (P@V out) = 8 banks
        ps_mix = octx.enter_context(tc.tile_pool(name="ps_mix", bufs=2, space="PSUM"))
        ps_st = octx.enter_context(tc.tile_pool(name="ps_st", bufs=2, space="PSUM"))
        ps_o = octx.enter_context(tc.tile_pool(name="ps_o", bufs=2, space="PSUM"))
        ppool = octx.enter_context(tc.tile_pool(name="p", bufs=16))
        rpool = octx.enter_context(tc.tile_pool(name="r", bufs=3))
        oupool = octx.enter_context(tc.tile_pool(name="ou", bufs=4))
        ypool = octx.enter_context(tc.tile_pool(name="y", bufs=2))

        def kn1_piece(m):
            """K projection n=1 for one m-tile (filler between ch0 stages)."""
            ps = ps_mix.tile([P, CH], FP, tag="mix", name=f"kn1_{m}")
            for kc in range(NT):
                nc.tensor.matmul(
                    ps, wk_sb[:, kc, m * P : (m + 1) * P],
                    xk_sb[:, kc, CH : 2 * CH],
                    start=(kc == 0), stop=(kc == NT - 1),
                )
            sl = slice(CH, 2 * CH)
            nc.scalar.activation(
                kT2_sb[0:DK, 2 * m, sl], ps[0:DK], AFT.Identity,
                bias=bk_sb[0:DK, m : m + 1],
            )
            nc.scalar.activation(
                kT2_sb[DK:P, 2 * m + 1, sl], ps[DK:P], AFT.Identity,
                bias=bk_sb[DK:P, m : m + 1],
            )

        def vm_piece(m):
            """V projection for key block m: out[s, hv]; lhsT = x^T tile."""
            pss = []
            for nh in range(NCH):
                ps = ps_mix.tile([P, CH], FP, tag="mix", name=f"v_{m}_{nh}")
                pss.append(ps)
            for kc in range(NT):
                for nh in range(NCH):
                    nc.tensor.matmul(
                        pss[nh], xv_sb[:, kc, m * P : (m + 1) * P],
                        wv_sb[:, kc, nh * CH : (nh + 1) * CH],
                        start=(kc == 0), stop=(kc == NT - 1),
                    )
            for nh in range(NCH):
                nc.vector.tensor_tensor(
                    v_sb[:, m, 8 * nh : 8 * (nh + 1), 0:DV],
                    pss[nh].rearrange("p (h v) -> p h v", v=DV),
                    bv_sb[:, nh * CH : (nh + 1) * CH].rearrange(
                        "p (h v) -> p h v", v=DV
                    ),
                    ALU.add,
                )

        # ---------------- attention + interleaved projections ------------
        # Chunk 0 (keys 0:512) needs only K n=0 and V blocks 0..3, so its
        # ACT-bound scores+exp interleave with the K n=1 matmuls as PE
        # filler; chunk 1 software-pipelines with chunk-0 output-projection
        # pieces. Filler is woven BETWEEN score groups so the in-order PE
        # never stalls on the score-PSUM ring while work waits behind it.
        state = {}

        def blocks_of(j):
            """(i, qoff, width, group, loc): key blocks for q-chunk j packed
            pairwise into score-PSUM groups. The last 4 blocks form the
            causal diagonal band; block 4j+k only covers q-cols 128k:512."""
            out = []
            for i in range(4 * j + 4):
                qoff = max(0, (i - 4 * j) * 128)
                g, first = divmod(i, 2)
                loc = 0 if first == 0 else out[-1][2]
                out.append((i, qoff, QC - qoff, g, loc))
            return out

        def st_sub(j, hp, sub, g):
            """Scores + exp + causal mask for one pair-group of one head."""
            qlo = j * QC
            st = state.setdefault((j, hp), {"pchs": {}})
            gb = [b for b in blocks_of(j) if b[3] == g]
            h = 2 * hp + sub
            stp = ps_st.tile([P, 1024], FP, tag="st", name=f"st_{j}_{hp}_{sub}_{g}")
            for i, qoff, w, _, loc in gb:
                nc.tensor.matmul(
                    stp[:, loc : loc + w],
                    kT2_sb[:, h, i * P : (i + 1) * P],
                    qT_sb[:, hp, qlo + qoff : qlo + QC],
                    start=True,
                    stop=True,
                )
            span = gb[-1][4] + gb[-1][2]
            pch = ppool.tile([P, 1024], BF, tag="p", name=f"p_{j}_{hp}_{sub}_{g}")
            nc.scalar.activation(pch[:, 0:span], stp[:, 0:span], AFT.Exp)
            if gb[0][0] >= 4 * j:  # band pair: combined triangular mask
                if gb[0][2] == QC:  # [512|384] pair
                    nc.vector.tensor_tensor(
                        pch[:, 0:640], pch[:, 0:640], cpack_sb[:, 0:640], ALU.mult
                    )
                else:  # [256|128] pair
                    nc.vector.tensor_tensor(
                        pch[:, 0:384], pch[:, 0:384], cpack_sb[:, 640:1024], ALU.mult
                    )
            st["pchs"][(sub, g)] = pch

        def pv_sub(j, hp, sub):
            st = state[(j, hp)]

            h = 2 * hp + sub
            blks = blocks_of(j)
            pos = ps_o.tile([P, QC], FP, tag="o", name=f"po_{j}_{hp}_{sub}")
            for i, qoff, w, g, loc in blks:
                nc.tensor.matmul(
                    pos[0 : DV + 1, qoff:QC],
                    v_sb[:, i, h, :],
                    st["pchs"][(sub, g)][:, loc : loc + w],
                    start=(i == 0),
                    stop=(i == len(blks) - 1),
                )
            # immediate PSUM->SBUF evacuation so the bank recycles fast
            ou = oupool.tile([DV, QC], FP, tag="ou", name=f"ou_{j}_{hp}_{sub}")
            nc.vector.tensor_copy(out=ou, in_=pos[0:DV])
            r1 = rpool.tile([1, QC], BF, tag="r1", name=f"r1_{j}_{hp}_{sub}")
            nc.vector.tensor_copy(out=r1, in_=pos[DV : DV + 1])
            st[("r1", sub)] = r1
            st[("ou", sub)] = ou

        def norm_stage(j, hp):
            """oT = ou / r: one K=2 matmul replicates both subs' denominator
            rows across the partition halves; fast reciprocal + mults on DVE."""
            st = state.pop((j, hp))
            qlo = j * QC
            for sub in (0, 1):
                hm = sub * DV
                rrep = ps_mix.tile(
                    [DV, CH], FP, tag="mix", name=f"rrep_{j}_{hp}_{sub}"
                )
                nc.tensor.matmul(
                    rrep[:, 0:QC], cpack_sb[0:1, P : P + DV], st[("r1", sub)],
                    start=True, stop=True,
                )
                rrinv = rpool.tile(
                    [DV, QC], FP, tag="rrinv", name=f"rrinv_{j}_{hp}_{sub}"
                )
                nc.vector.reciprocal_approx_fast(rrinv, rrep[:, 0:QC])
                nc.vector.tensor_tensor(
                    oT_sb[hm : hm + DV, hp, qlo : qlo + QC],
                    st[("ou", sub)],
                    rrinv,
                    ALU.mult,
                )

        def outproj_piece(m, n):
            py = ps_mix.tile([P, CH], FP, tag="mix", name=f"py_{m}_{n}")
            for kc in range(NT):
                nc.tensor.matmul(
                    py,
                    oT_sb[:, kc, m * P : (m + 1) * P],
                    wo_sb[:, kc, n * CH : (n + 1) * CH],
                    start=(kc == 0),
                    stop=(kc == NT - 1),
                )
            yt = ypool.tile([P, CH], FP, tag="y")
            nc.vector.tensor_tensor(
                yt, py, borep_sb[:, n * CH : (n + 1) * CH], ALU.add
            )
            nc.sync.dma_start(y_r[:, m, n * CH : (n + 1) * CH], yt)

        # chunk 0 with K n=1 and V-projection pieces woven between score
        # groups (V's input DMAs land under the early steps' compute)
        fillers = [
            ("kn1", 0), ("vm", 0), ("vm", 1), ("kn1", 1), ("vm", 2),
            ("vm", 3), ("kn1", 2), ("kn1", 3), ("kn1", 4), ("kn1", 5),
            ("kn1", 6), ("kn1", 7), ("vm", 4), ("vm", 5), ("vm", 6),
            ("vm", 7),
        ]

        def filler():
            if fillers:
                kind, m = fillers.pop(0)
                (kn1_piece if kind == "kn1" else vm_piece)(m)

        for hp in range(H // 2):
            st_sub(0, hp, 0, 0)
            st_sub(0, hp, 0, 1)
            filler()
            if hp >= 3:
                pv_sub(0, hp - 3, 0)
            st_sub(0, hp, 1, 0)
            st_sub(0, hp, 1, 1)
            filler()
            if hp >= 3:
                pv_sub(0, hp - 3, 1)
            if hp >= 4:
                norm_stage(0, hp - 4)
        # drain chunk 0 (remaining V pieces interleave as filler)
        for hp in (5, 6, 7):
            for sub in (0, 1):
                pv_sub(0, hp, sub)
            filler()
            norm_stage(0, hp - 1)
        filler()
        norm_stage(0, 7)

        op_queue = [(m, n) for m in range(4) for n in range(NCH)]

        # chunk 1 software-pipelined with chunk-0 output-projection filler
        for hp in range(H // 2):
            st_sub(1, hp, 0, 0)
            st_sub(1, hp, 0, 1)
            if hp >= 1:
                pv_sub(1, hp - 1, 0)
            st_sub(1, hp, 0, 2)
            st_sub(1, hp, 0, 3)
            if hp >= 1:
                pv_sub(1, hp - 1, 1)
            st_sub(1, hp, 1, 0)
            st_sub(1, hp, 1, 1)
            outproj_piece(*op_queue.pop(0))
            st_sub(1, hp, 1, 2)
            st_sub(1, hp, 1, 3)
            if hp >= 2:
                norm_stage(1, hp - 2)
        for sub in (0, 1):
            pv_sub(1, 7, sub)
        norm_stage(1, 6)
        norm_stage(1, 7)
        op_queue += [(m, n) for m in range(4, NT) for n in range(NCH)]
        while op_queue:
            outproj_piece(*op_queue.pop(0))

    nc.finalize()
    return nc


_NC_CACHE = None


def _get_nc() -> bass.Bass:
    global _NC_CACHE
    if _NC_CACHE is None:
        _NC_CACHE = build_nc()
    return _NC_CACHE


def prep_shared(Wq, bq, Wk, bk, Wv, bv, Wo, bo):
    """Host-side packing of weights/biases (shared by all cores)."""
    scale = 1.0 / math.sqrt(DK)
    Wq = np.asarray(Wq, np.float32)
    Wk = np.asarray(Wk, np.float32)
    Wv = np.asarray(Wv, np.float32)
    Wo = np.asarray(Wo, np.float32)
    out = {
        "wq": np.ascontiguousarray(
            (Wq.transpose(1, 0, 2).reshape(C, H * DK) * scale).astype(BF_NP)
        ),
        "wk": np.ascontiguousarray(
            Wk.transpose(1, 0, 2).reshape(C, H * DK).astype(BF_NP)
        ),
        "wv": np.ascontiguousarray(
            Wv.transpose(1, 0, 2).reshape(C, H * DV).astype(BF_NP)
        ),
        "wo": Wo.astype(BF_NP),
        "bq": np.ascontiguousarray(
            (np.asarray(bq, np.float32).reshape(H * DK) * scale)
            .reshape(NT, P)
            .T.astype(np.float32)
        ),
        "bk": np.ascontiguousarray(
            np.asarray(bk, np.float32).reshape(NT, P).T.astype(np.float32)
        ),
        "bv": np.ascontiguousarray(
            np.broadcast_to(
                np.asarray(bv, np.float32).reshape(1, H * DV), (P, H * DV)
            ).astype(BF_NP)
        ),
        "bo": np.ascontiguousarray(
            np.asarray(bo, np.float32).reshape(1, C).astype(BF_NP)
        ),
    }
    return out


def prep_core(q_embs_b, k_embs_b, v_embs_b):
    return {
        "xq": np.ascontiguousarray(np.asarray(q_embs_b, np.float32).T.astype(BF_NP)),
        "xk": np.ascontiguousarray(np.asarray(k_embs_b, np.float32).T.astype(BF_NP)),
        "xv": np.ascontiguousarray(np.asarray(v_embs_b, np.float32).T.astype(BF_NP)),
    }


def kernel(q_embs, k_embs, v_embs, Wq, bq, Wk, bk, Wv, bv, Wo, bo, **run_kwargs):
    nc = _get_nc()
    shared = prep_shared(Wq, bq, Wk, bk, Wv, bv, Wo, bo)
    q_embs = np.asarray(q_embs, np.float32)
    k_embs = np.asarray(k_embs, np.float32)
    v_embs = np.asarray(v_embs, np.float32)
    in_maps = []
    for b in range(B):
        m = dict(shared)
        m.update(prep_core(q_embs[b], k_embs[b], v_embs[b]))
        in_maps.append(m)
    res = run_bass_kernel_spmd(nc, in_maps, core_ids=list(range(B)), **run_kwargs)
    out = np.stack([res.results[i]["y"] for i in range(B)], axis=0)
    if run_kwargs:
        kernel.last_results = res
    return out


if __name__ == "__main__":
    rng = np.random.default_rng(0)
    inputs = {
        "q_embs": rng.standard_normal((B, S, C), np.float32),
        "k_embs": rng.standard_normal((B, S, C), np.float32),
        "v_embs": rng.standard_normal((B, S, C), np.float32),
        "Wq": rng.standard_normal((H, C, DK), np.float32) * 0.02,
        "bq": np.zeros((H, DK), np.float32),
        "Wk": rng.standard_normal((H, C, DK), np.float32) * 0.02,
        "bk": np.zeros((H, DK), np.float32),
        "Wv": rng.standard_normal((H, C, DV), np.float32) * 0.02,
        "bv": np.zeros((H, DV), np.float32),
        "Wo": rng.standard_normal((H * DV, C), np.float32) * 0.02,
        "bo": np.zeros((C,), np.float32),
    }
    out = kernel(**inputs)
    print(out.shape, out.dtype)


# revision 35
# speedup vs baseline: 1.2725x; 1.0553x over previous
"""Multi-head causal attention (B=8, S=1024, C=1024, H=16, dk=dv=64) on 8 trn2 cores.

Sharding: data-parallel over batch. Each NeuronCore processes one batch element
end-to-end (projections + attention + output projection); no collectives.

Per-core layout:
  inputs (host-prepped): xq/xk/xv = X^T [C, S] bf16, packed weights
  wq/wk [C, H*DK] (wq pre-scaled by 1/sqrt(dk)), wv [C, H*DV], wo [H*DV, C],
  biases in per-partition / replicated layouts.

  QT = wq.T @ xq  -> [H*DK, S]   (head-major rows)
  KT = wk.T @ xk  -> [H*DK, S]   (zero-padded per-head tiles for K=128 St)
  V  = xv.T @ wv  -> [S, H*DV]   (+ appended ones column per head)

  Attention runs in exact-causal q-chunks of 256: for q-chunk j only key
  blocks 0..2j+1 are computed (the last block only for its live 128 q-cols).
  Score PSUM for up to 4 key blocks is merged so one ACT Exp instruction
  covers ~1K columns; the causal diagonal is a post-exp triangular multiply
  on DVE. P@[V|1] accumulates O^T rows plus the softmax denominator row; the
  denominator is replicated across partitions with a tiny K=2 matmul and
  inverted with the fast DVE reciprocal, keeping the ACT engine exclusively
  on Exp. Output projection per finished 128-q-row tile.
"""

import math
import os
import sys

import numpy as np

try:
    import concourse.bass as bass
except ImportError:  # make concourse importable in a bare grading dir
    for _p in ("/opt/trn_rl_repo", os.path.expanduser("~/.axon_site/_ro/trn_rl_repo")):
        if os.path.isdir(_p) and _p not in sys.path:
            sys.path.insert(0, _p)
    import concourse.bass as bass

from contextlib import ExitStack

import ml_dtypes

import concourse.mybir as mybir
import concourse.tile as tile
from concourse import bacc
from concourse.bass_utils import run_bass_kernel_spmd

def _setup_act_tables():
    """Pin the ACT function table to the set that covers exp+ln+identity+copy
    so the kernel never reloads LUTs mid-flight. Both the bacc-side pass and
    walrus must see the same (reordered) act_info.json."""
    import json
    import shutil
    import tempfile

    import concourse.hw_specs as hw_specs
    from concourse import bacc as _bacc

    if os.environ.get("BASS_ACT_ROOT_JSON_PATH"):
        return  # already configured
    from neuronxcc.driver.Job import Job

    orig = os.path.join(
        Job.getPackageDir(), "pwp", "pwp_bin_trainium", "act_info.json"
    )
    assert os.path.isfile(orig), orig
    dst = os.path.join(tempfile.gettempdir(), "mha_act_tables")
    if not os.path.isdir(dst):
        tmp = dst + ".tmp"
        shutil.rmtree(tmp, ignore_errors=True)
        shutil.copytree(os.path.dirname(orig), tmp)
        with open(os.path.join(tmp, "act_info.json")) as f:
            info = json.load(f)
        sets = info["act_func_sets"]
        want = [s for s in sets if s["name"] == "natural_log_exp_and_others"]
        rest = [s for s in sets if s["name"] != "natural_log_exp_and_others"]
        info["act_func_sets"] = want + rest
        with open(os.path.join(tmp, "act_info.json"), "w") as f:
            json.dump(info, f)
        os.replace(tmp, dst)
    path = os.path.join(dst, "act_info.json")
    os.environ["BASS_ACT_ROOT_JSON_PATH"] = path

    def patched(module_arch):
        with open(path) as af:
            act_info = json.load(af)
        return {
            ent["name"]: {
                mybir.ActivationFunctionType.from_pwp(v) for v in ent["act"].keys()
            }
            for ent in act_info["act_func_sets"]
        }

    hw_specs.get_activation_tables = patched
    _bacc.get_activation_tables = patched
    from concourse import bass_interp as _bi

    _bi.get_activation_tables = patched


B, S, C = 8, 1024, 1024
H, DK, DV = 16, 64, 64
P = 128
NT = 8  # number of 128-tiles along S / C / H*DK
CH = 512  # projection free-dim chunk (one PSUM bank of fp32)
NCH = S // CH
QC = 512  # attention q-chunk
NQC = S // QC


FP = mybir.dt.float32
BF = mybir.dt.bfloat16
BF_NP = ml_dtypes.bfloat16
AFT = mybir.ActivationFunctionType
ALU = mybir.AluOpType


def build_nc() -> bass.Bass:
    _setup_act_tables()
    nc = bacc.Bacc()

    xq = nc.dram_tensor("xq", [C, S], BF, kind="ExternalInput")
    xk = nc.dram_tensor("xk", [C, S], BF, kind="ExternalInput")
    xv = nc.dram_tensor("xv", [C, S], BF, kind="ExternalInput")
    wq = nc.dram_tensor("wq", [C, H * DK], BF, kind="ExternalInput")
    wk = nc.dram_tensor("wk", [C, H * DK], BF, kind="ExternalInput")
    wv = nc.dram_tensor("wv", [C, H * DV], BF, kind="ExternalInput")
    wo = nc.dram_tensor("wo", [H * DV, C], BF, kind="ExternalInput")
    bqd = nc.dram_tensor("bq", [P, NT], FP, kind="ExternalInput")
    bkd = nc.dram_tensor("bk", [P, NT], FP, kind="ExternalInput")
    bvd = nc.dram_tensor("bv", [P, H * DV], BF, kind="ExternalInput")
    bod = nc.dram_tensor("bo", [1, C], BF, kind="ExternalInput")
    y = nc.dram_tensor("y", [S, C], FP, kind="ExternalOutput")

    # constants baked into the NEFF, one bf16 tensor (single DMA):
    # cols 0:640   = causal mask for a band-block pair stored as [512|384]:
    #                [tri | ones(384) | tri]  (tri[t,q] = 1 iff t<=q)
    # cols 640:1024 = mask for a pair stored as [256|128]: [tri | ones | tri]
    tri = np.triu(np.ones((P, P), np.float32))
    ones = np.ones((P, P), np.float32)
    pat2 = np.zeros((P, P), np.float32)
    pat2[0, 0:DV] = 1.0
    pat2[32, DV:P] = 1.0
    cpack_np = np.concatenate(
        [tri, ones, ones, ones, tri, tri, ones, tri, pat2], axis=1
    )
    cpack_d = nc.inline_tensor(cpack_np.astype(BF_NP), "cpack")

    xq_r = xq.rearrange("(ko p) s -> p ko s", p=P)
    xk_r = xk.rearrange("(ko p) s -> p ko s", p=P)
    xv_r = xv.rearrange("(ko p) s -> p ko s", p=P)
    wq_r = wq.rearrange("(ko p) m -> p ko m", p=P)
    wk_r = wk.rearrange("(ko p) m -> p ko m", p=P)
    wv_r = wv.rearrange("(ko p) m -> p ko m", p=P)
    wo_r = wo.rearrange("(ko p) c -> p ko c", p=P)
    y_r = y.rearrange("(mo p) c -> p mo c", p=P)

    with tile.TileContext(nc) as tc, ExitStack() as octx:
        const = octx.enter_context(tc.tile_pool(name="const", bufs=1))
        qk = octx.enter_context(tc.tile_pool(name="qk", bufs=1))
        opool = octx.enter_context(tc.tile_pool(name="oT", bufs=1))

        qT_sb = qk.tile([P, NT, S], BF, tag="qT")
        kT2_sb = qk.tile([P, H, S], BF, tag="kT2")
        v_sb = qk.tile([P, NT, H, DV + 1], BF, tag="v")
        oT_sb = opool.tile([P, NT, S], BF, tag="oT")

        # ---------------- pools + DMAs ----------------
        wpool = octx.enter_context(tc.tile_pool(name="wqkv", bufs=2))
        xpool = octx.enter_context(tc.tile_pool(name="xin", bufs=2))

        wq_sb = wpool.tile([P, NT, H * DK], BF, tag="w", name="wq_sb")
        xq_sb = xpool.tile([P, NT, S], BF, tag="x", name="xq_sb")
        wk_sb = wpool.tile([P, NT, H * DK], BF, tag="w", name="wk_sb")
        xk_sb = xpool.tile([P, NT, S], BF, tag="x", name="xk_sb")
        # Q inputs + small consts first so the PE starts ASAP
        nc.sync.dma_start(wq_sb[:, 0], wq_r[:, 0])
        nc.sync.dma_start(xq_sb[:, 0], xq_r[:, 0])
        bq_sb = const.tile([P, NT], FP, tag="bq")
        nc.sync.dma_start(bq_sb, bqd[:])
        bk_sb = const.tile([P, NT], FP, tag="bk")
        nc.sync.dma_start(bk_sb, bkd[:])
        cpack_sb = const.tile([P, 1152], BF, tag="cpack")
        nc.sync.dma_start(cpack_sb, cpack_d[:])
        for kc in range(1, NT):
            nc.sync.dma_start(wq_sb[:, kc], wq_r[:, kc])
            nc.sync.dma_start(xq_sb[:, kc], xq_r[:, kc])
        for kc in range(NT):
            nc.sync.dma_start(wk_sb[:, kc], wk_r[:, kc])
            nc.sync.dma_start(xk_sb[:, kc], xk_r[:, kc])

        bo_sb = const.tile([1, C], BF, tag="bo")
        nc.sync.dma_start(bo_sb, bod[:])
        borep_sb = const.tile([P, C], BF, tag="borep")
        nc.gpsimd.partition_broadcast(borep_sb, bo_sb)
        bv_sb = const.tile([P, H * DV], BF, tag="bv")
        nc.sync.dma_start(bv_sb, bvd[:])

        nc.vector.memset(v_sb[:, :, :, DV], 1.0)
        # zero the unused half of each head's K^T tile so St matmuls can
        # contract over the full 128 partitions (the zero lhsT rows
        # nullify the other head's Q rows)
        for h in range(H):
            hz = DK if h % 2 == 0 else 0
            nc.gpsimd.memset(kT2_sb[hz : hz + DK, h, :], 0.0)

        # V inputs reuse Q's buffers (free once Q's matmuls finish); the
        # output-projection weight reuses K's buffer after K's last matmul
        wv_sb = wpool.tile([P, NT, H * DV], BF, tag="w", name="wv_sb")
        xv_sb = xpool.tile([P, NT, S], BF, tag="x", name="xv_sb")
        for kc in range(NT):
            nc.sync.dma_start(wv_sb[:, kc], wv_r[:, kc])
            nc.sync.dma_start(xv_sb[:, kc], xv_r[:, kc])
        wo_sb = wpool.tile([P, NT, C], BF, tag="w", name="wo_sb")
        nc.sync.dma_start(wo_sb, wo_r)

        # ---------------- Q + K(n=0) projections ----------------
        # out[hk, s]; lhsT = w tile [c, hk], rhs = x^T [c, s]; kc-outer so
        # matmuls chase the input DMAs; evacuation (+bias) on the idle ACT.
        def qk_evac_m(ps, m, n, b_sb, out_sb):
            sl = slice(n * CH, (n + 1) * CH)
            if out_sb is not None:
                nc.vector.tensor_scalar_add(
                    out_sb[:, m, sl], ps, b_sb[:, m : m + 1]
                )
            else:  # K^T: split the head pair into per-head tiles
                nc.vector.tensor_scalar_add(
                    kT2_sb[0:DK, 2 * m, sl], ps[0:DK], b_sb[0:DK, m : m + 1]
                )
                nc.vector.tensor_scalar_add(
                    kT2_sb[DK:P, 2 * m + 1, sl], ps[DK:P], b_sb[DK:P, m : m + 1]
                )

        with ExitStack() as ictx:
            psproj = ictx.enter_context(
                tc.tile_pool(name="psproj", bufs=8, space="PSUM")
            )
            # Q n=0: kc-outer so matmuls chase the input DMAs
            psums = []
            for kc in range(NT):
                for m in range(NT):
                    if kc == 0:
                        psums.append(psproj.tile(
                            [P, CH], FP, tag="proj", name=f"proj_ps_{m}"))
                    nc.tensor.matmul(
                        psums[m],
                        wq_sb[:, kc, m * P : (m + 1) * P],
                        xq_sb[:, kc, 0:CH],
                        start=(kc == 0), stop=(kc == NT - 1),
                    )
            for m in range(NT):
                qk_evac_m(psums[m], m, 0, bq_sb, qT_sb)
            # Q n=1 / K n=0: m-outer so each tile's evacuation overlaps the
            # next tile's matmuls (keeps the DVE queue clear of bursts)
            for w_sb, x_sb, b_sb, out_sb, n in (
                (wq_sb, xq_sb, bq_sb, qT_sb, 1),
                (wk_sb, xk_sb, bk_sb, None, 0),
            ):
                for m in range(NT):
                    ps = psproj.tile([P, CH], FP, tag="proj", name=f"proj_ps_{m}")
                    for kc in range(NT):
                        nc.tensor.matmul(
                            ps,
                            w_sb[:, kc, m * P : (m + 1) * P],
                            x_sb[:, kc, n * CH : (n + 1) * CH],
                            start=(kc == 0), stop=(kc == NT - 1),
                        )
                    qk_evac_m(ps, m, n, b_sb, out_sb)

        # remaining PSUM pools: 3 (mix) + 3 (scores) + 2 (P@V out) = 8 banks
        ps_mix = octx.enter_context(tc.tile_pool(name="ps_mix", bufs=2, space="PSUM"))
        ps_st = octx.enter_context(tc.tile_pool(name="ps_st", bufs=2, space="PSUM"))
        ps_o = octx.enter_context(tc.tile_pool(name="ps_o", bufs=2, space="PSUM"))
        ppool = octx.enter_context(tc.tile_pool(name="p", bufs=16))
        rpool = octx.enter_context(tc.tile_pool(name="r", bufs=3))
        oupool = octx.enter_context(tc.tile_pool(name="ou", bufs=4))
        ypool = octx.enter_context(tc.tile_pool(name="y", bufs=2))

        def kn1_piece(m):
            """K projection n=1 for one m-tile (filler between ch0 stages)."""
            ps = ps_mix.tile([P, CH], FP, tag="mix", name=f"kn1_{m}")
            for kc in range(NT):
                nc.tensor.matmul(
                    ps, wk_sb[:, kc, m * P : (m + 1) * P],
                    xk_sb[:, kc, CH : 2 * CH],
                    start=(kc == 0), stop=(kc == NT - 1),
                )
            sl = slice(CH, 2 * CH)
            nc.scalar.activation(
                kT2_sb[0:DK, 2 * m, sl], ps[0:DK], AFT.Identity,
                bias=bk_sb[0:DK, m : m + 1],
            )
            nc.scalar.activation(
                kT2_sb[DK:P, 2 * m + 1, sl], ps[DK:P], AFT.Identity,
                bias=bk_sb[DK:P, m : m + 1],
            )

        def vm_piece(m):
            """V projection for key block m: out[s, hv]; lhsT = x^T tile."""
            pss = []
            for nh in range(NCH):
                ps = ps_mix.tile([P, CH], FP, tag="mix", name=f"v_{m}_{nh}")
                pss.append(ps)
            for kc in range(NT):
                for nh in range(NCH):
                    nc.tensor.matmul(
                        pss[nh], xv_sb[:, kc, m * P : (m + 1) * P],
                        wv_sb[:, kc, nh * CH : (nh + 1) * CH],
                        start=(kc == 0), stop=(kc == NT - 1),
                    )
            for nh in range(NCH):
                nc.vector.tensor_tensor(
                    v_sb[:, m, 8 * nh : 8 * (nh + 1), 0:DV],
                    pss[nh].rearrange("p (h v) -> p h v", v=DV),
                    bv_sb[:, nh * CH : (nh + 1) * CH].rearrange(
                        "p (h v) -> p h v", v=DV
                    ),
                    ALU.add,
                )

        # ---------------- attention + interleaved projections ------------
        # Chunk 0 (keys 0:512) needs only K n=0 and V blocks 0..3, so its
        # ACT-bound scores+exp interleave with the K n=1 matmuls as PE
        # filler; chunk 1 software-pipelines with chunk-0 output-projection
        # pieces. Filler is woven BETWEEN score groups so the in-order PE
        # never stalls on the score-PSUM ring while work waits behind it.
        state = {}

        def blocks_of(j):
            """(i, qoff, width, group, loc): key blocks for q-chunk j packed
            pairwise into score-PSUM groups. The last 4 blocks form the
            causal diagonal band; block 4j+k only covers q-cols 128k:512."""
            out = []
            for i in range(4 * j + 4):
                qoff = max(0, (i - 4 * j) * 128)
                g, first = divmod(i, 2)
                loc = 0 if first == 0 else out[-1][2]
                out.append((i, qoff, QC - qoff, g, loc))
            return out

        def st_sub(j, hp, sub, g):
            """Scores + exp + causal mask for one pair-group of one head."""
            qlo = j * QC
            st = state.setdefault((j, hp), {"pchs": {}})
            gb = [b for b in blocks_of(j) if b[3] == g]
            h = 2 * hp + sub
            stp = ps_st.tile([P, 1024], FP, tag="st", name=f"st_{j}_{hp}_{sub}_{g}")
            for i, qoff, w, _, loc in gb:
                nc.tensor.matmul(
                    stp[:, loc : loc + w],
                    kT2_sb[:, h, i * P : (i + 1) * P],
                    qT_sb[:, hp, qlo + qoff : qlo + QC],
                    start=True,
                    stop=True,
                )
            span = gb[-1][4] + gb[-1][2]
            pch = ppool.tile([P, 1024], BF, tag="p", name=f"p_{j}_{hp}_{sub}_{g}")
            nc.scalar.activation(pch[:, 0:span], stp[:, 0:span], AFT.Exp)
            if gb[0][0] >= 4 * j:  # band pair: combined triangular mask
                if gb[0][2] == QC:  # [512|384] pair
                    nc.vector.tensor_tensor(
                        pch[:, 0:640], pch[:, 0:640], cpack_sb[:, 0:640], ALU.mult
                    )
                else:  # [256|128] pair
                    nc.vector.tensor_tensor(
                        pch[:, 0:384], pch[:, 0:384], cpack_sb[:, 640:1024], ALU.mult
                    )
            st["pchs"][(sub, g)] = pch

        def pv_sub(j, hp, sub):
            st = state[(j, hp)]

            h = 2 * hp + sub
            blks = blocks_of(j)
            pos = ps_o.tile([P, QC], FP, tag="o", name=f"po_{j}_{hp}_{sub}")
            for i, qoff, w, g, loc in blks:
                nc.tensor.matmul(
                    pos[0 : DV + 1, qoff:QC],
                    v_sb[:, i, h, :],
                    st["pchs"][(sub, g)][:, loc : loc + w],
                    start=(i == 0),
                    stop=(i == len(blks) - 1),
                )
            # immediate PSUM->SBUF evacuation so the bank recycles fast
            ou = oupool.tile([DV, QC], FP, tag="ou", name=f"ou_{j}_{hp}_{sub}")
            nc.vector.tensor_copy(out=ou, in_=pos[0:DV])
            r1 = rpool.tile([1, QC], BF, tag="r1", name=f"r1_{j}_{hp}_{sub}")
            nc.vector.tensor_copy(out=r1, in_=pos[DV : DV + 1])
            st[("r1", sub)] = r1
            st[("ou", sub)] = ou

        def norm_stage(j, hp):
            """oT = ou / r: one K=2 matmul replicates both subs' denominator
            rows across the partition halves; fast reciprocal + mults on DVE."""
            st = state.pop((j, hp))
            qlo = j * QC
            for sub in (0, 1):
                hm = sub * DV
                rrep = ps_mix.tile(
                    [DV, CH], FP, tag="mix", name=f"rrep_{j}_{hp}_{sub}"
                )
                nc.tensor.matmul(
                    rrep[:, 0:QC], cpack_sb[0:1, P : P + DV], st[("r1", sub)],
                    start=True, stop=True,
                )
                rrinv = rpool.tile(
                    [DV, QC], FP, tag="rrinv", name=f"rrinv_{j}_{hp}_{sub}"
                )
                nc.vector.reciprocal_approx_fast(rrinv, rrep[:, 0:QC])
                nc.vector.tensor_tensor(
                    oT_sb[hm : hm + DV, hp, qlo : qlo + QC],
                    st[("ou", sub)],
                    rrinv,
                    ALU.mult,
                )

        def outproj_piece(m, n, pool=None):
            if pool is None:
                py = ps_mix.tile([P, CH], FP, tag="mix", name=f"py_{m}_{n}")
            else:
                py = pool.tile([P, 1024], FP, tag="st", name=f"py_{m}_{n}")[:, 0:CH]
            for kc in range(NT):
                nc.tensor.matmul(
                    py,
                    oT_sb[:, kc, m * P : (m + 1) * P],
                    wo_sb[:, kc, n * CH : (n + 1) * CH],
                    start=(kc == 0),
                    stop=(kc == NT - 1),
                )
            yt = ypool.tile([P, CH], FP, tag="y")
            nc.vector.tensor_tensor(
                yt, py, borep_sb[:, n * CH : (n + 1) * CH], ALU.add
            )
            nc.sync.dma_start(y_r[:, m, n * CH : (n + 1) * CH], yt)

        # chunk 0 with K n=1 and V-projection pieces woven between score
        # groups (V's input DMAs land under the early steps' compute)
        fillers = [
            ("kn1", 0), ("vm", 0), ("vm", 1), ("kn1", 1), ("vm", 2),
            ("vm", 3), ("kn1", 2), ("kn1", 3), ("kn1", 4), ("kn1", 5),
            ("kn1", 6), ("kn1", 7), ("vm", 4), ("vm", 5), ("vm", 6),
            ("vm", 7),
        ]

        def filler():
            if fillers:
                kind, m = fillers.pop(0)
                (kn1_piece if kind == "kn1" else vm_piece)(m)

        for hp in range(H // 2):
            st_sub(0, hp, 0, 0)
            st_sub(0, hp, 0, 1)
            filler()
            if hp >= 3:
                pv_sub(0, hp - 3, 0)
            st_sub(0, hp, 1, 0)
            st_sub(0, hp, 1, 1)
            filler()
            if hp >= 3:
                pv_sub(0, hp - 3, 1)
            if hp >= 4:
                norm_stage(0, hp - 4)
        # drain chunk 0 (remaining V pieces interleave as filler)
        for hp in (5, 6, 7):
            for sub in (0, 1):
                pv_sub(0, hp, sub)
            filler()
            norm_stage(0, hp - 1)
        filler()
        norm_stage(0, 7)

        op_queue = [(m, n) for m in range(4) for n in range(NCH)]

        # chunk 1 software-pipelined with chunk-0 output-projection filler
        for hp in range(H // 2):
            st_sub(1, hp, 0, 0)
            st_sub(1, hp, 0, 1)
            if hp >= 1:
                pv_sub(1, hp - 1, 0)
            st_sub(1, hp, 0, 2)
            st_sub(1, hp, 0, 3)
            if hp >= 1:
                pv_sub(1, hp - 1, 1)
            st_sub(1, hp, 1, 0)
            st_sub(1, hp, 1, 1)
            outproj_piece(*op_queue.pop(0))
            st_sub(1, hp, 1, 2)
            st_sub(1, hp, 1, 3)
            if hp >= 2:
                norm_stage(1, hp - 2)
        for sub in (0, 1):
            pv_sub(1, 7, sub)
        norm_stage(1, 6)
        norm_stage(1, 7)
        op_queue += [(m, n) for m in range(4, NT) for n in range(NCH)]
        alt = 0
        while op_queue:
            outproj_piece(*op_queue.pop(0), pool=(ps_st if alt % 2 else None))
            alt += 1

    nc.finalize()
    return nc


_NC_CACHE = None


def _get_nc() -> bass.Bass:
    global _NC_CACHE
    if _NC_CACHE is None:
        _NC_CACHE = build_nc()
    return _NC_CACHE


def prep_shared(Wq, bq, Wk, bk, Wv, bv, Wo, bo):
    """Host-side packing of weights/biases (shared by all cores)."""
    scale = 1.0 / math.sqrt(DK)
    Wq = np.asarray(Wq, np.float32)
    Wk = np.asarray(Wk, np.float32)
    Wv = np.asarray(Wv, np.float32)
    Wo = np.asarray(Wo, np.float32)
    out = {
        "wq": np.ascontiguousarray(
            (Wq.transpose(1, 0, 2).reshape(C, H * DK) * scale).astype(BF_NP)
        ),
        "wk": np.ascontiguousarray(
            Wk.transpose(1, 0, 2).reshape(C, H * DK).astype(BF_NP)
        ),
        "wv": np.ascontiguousarray(
            Wv.transpose(1, 0, 2).reshape(C, H * DV).astype(BF_NP)
        ),
        "wo": Wo.astype(BF_NP),
        "bq": np.ascontiguousarray(
            (np.asarray(bq, np.float32).reshape(H * DK) * scale)
            .reshape(NT, P)
            .T.astype(np.float32)
        ),
        "bk": np.ascontiguousarray(
            np.asarray(bk, np.float32).reshape(NT, P).T.astype(np.float32)
        ),
        "bv": np.ascontiguousarray(
            np.broadcast_to(
                np.asarray(bv, np.float32).reshape(1, H * DV), (P, H * DV)
            ).astype(BF_NP)
        ),
        "bo": np.ascontiguousarray(
            np.asarray(bo, np.float32).reshape(1, C).astype(BF_NP)
        ),
    }
    return out


def prep_core(q_embs_b, k_embs_b, v_embs_b):
    return {
        "xq": np.ascontiguousarray(np.asarray(q_embs_b, np.float32).T.astype(BF_NP)),
        "xk": np.ascontiguousarray(np.asarray(k_embs_b, np.float32).T.astype(BF_NP)),
        "xv": np.ascontiguousarray(np.asarray(v_embs_b, np.float32).T.astype(BF_NP)),
    }


def kernel(q_embs, k_embs, v_embs, Wq, bq, Wk, bk, Wv, bv, Wo, bo, **run_kwargs):
    nc = _get_nc()
    shared = prep_shared(Wq, bq, Wk, bk, Wv, bv, Wo, bo)
    q_embs = np.asarray(q_embs, np.float32)
    k_embs = np.asarray(k_embs, np.float32)
    v_embs = np.asarray(v_embs, np.float32)
    in_maps = []
    for b in range(B):
        m = dict(shared)
        m.update(prep_core(q_embs[b], k_embs[b], v_embs[b]))
        in_maps.append(m)
    res = run_bass_kernel_spmd(nc, in_maps, core_ids=list(range(B)), **run_kwargs)
    out = np.stack([res.results[i]["y"] for i in range(B)], axis=0)
    if run_kwargs:
        kernel.last_results = res
    return out


if __name__ == "__main__":
    rng = np.random.default_rng(0)
    inputs = {
        "q_embs": rng.standard_normal((B, S, C), np.float32),
        "k_embs": rng.standard_normal((B, S, C), np.float32),
        "v_embs": rng.standard_normal((B, S, C), np.float32),
        "Wq": rng.standard_normal((H, C, DK), np.float32) * 0.02,
        "bq": np.zeros((H, DK), np.float32),
        "Wk": rng.standard_normal((H, C, DK), np.float32) * 0.02,
        "bk": np.zeros((H, DK), np.float32),
        "Wv": rng.standard_normal((H, C, DV), np.float32) * 0.02,
        "bv": np.zeros((H, DV), np.float32),
        "Wo": rng.standard_normal((H * DV, C), np.float32) * 0.02,
        "bo": np.zeros((C,), np.float32),
    }
    out = kernel(**inputs)
    print(out.shape, out.dtype)


# revision 36
# speedup vs baseline: 1.3070x; 1.0271x over previous
"""Multi-head causal attention (B=8, S=1024, C=1024, H=16, dk=dv=64) on 8 trn2 cores.

Sharding: data-parallel over batch. Each NeuronCore processes one batch element
end-to-end (projections + attention + output projection); no collectives.

Per-core layout:
  inputs (host-prepped): xq/xk/xv = X^T [C, S] bf16, packed weights
  wq/wk [C, H*DK] (wq pre-scaled by 1/sqrt(dk)), wv [C, H*DV], wo [H*DV, C],
  biases in per-partition / replicated layouts.

  QT = wq.T @ xq  -> [H*DK, S]   (head-major rows)
  KT = wk.T @ xk  -> [H*DK, S]   (zero-padded per-head tiles for K=128 St)
  V  = xv.T @ wv  -> [S, H*DV]   (+ appended ones column per head)

  Attention runs in exact-causal q-chunks of 256: for q-chunk j only key
  blocks 0..2j+1 are computed (the last block only for its live 128 q-cols).
  Score PSUM for up to 4 key blocks is merged so one ACT Exp instruction
  covers ~1K columns; the causal diagonal is a post-exp triangular multiply
  on DVE. P@[V|1] accumulates O^T rows plus the softmax denominator row; the
  denominator is replicated across partitions with a tiny K=2 matmul and
  inverted with the fast DVE reciprocal, keeping the ACT engine exclusively
  on Exp. Output projection per finished 128-q-row tile.
"""

import math
import os
import sys

import numpy as np

try:
    import concourse.bass as bass
except ImportError:  # make concourse importable in a bare grading dir
    for _p in ("/opt/trn_rl_repo", os.path.expanduser("~/.axon_site/_ro/trn_rl_repo")):
        if os.path.isdir(_p) and _p not in sys.path:
            sys.path.insert(0, _p)
    import concourse.bass as bass

from contextlib import ExitStack

import ml_dtypes

import concourse.mybir as mybir
import concourse.tile as tile
from concourse import bacc
from concourse.bass_utils import run_bass_kernel_spmd

def _setup_act_tables():
    """Pin the ACT function table to the set that covers exp+ln+identity+copy
    so the kernel never reloads LUTs mid-flight. Both the bacc-side pass and
    walrus must see the same (reordered) act_info.json."""
    import json
    import shutil
    import tempfile

    import concourse.hw_specs as hw_specs
    from concourse import bacc as _bacc

    if os.environ.get("BASS_ACT_ROOT_JSON_PATH"):
        return  # already configured
    from neuronxcc.driver.Job import Job

    orig = os.path.join(
        Job.getPackageDir(), "pwp", "pwp_bin_trainium", "act_info.json"
    )
    assert os.path.isfile(orig), orig
    dst = os.path.join(tempfile.gettempdir(), "mha_act_tables")
    if not os.path.isdir(dst):
        tmp = dst + ".tmp"
        shutil.rmtree(tmp, ignore_errors=True)
        shutil.copytree(os.path.dirname(orig), tmp)
        with open(os.path.join(tmp, "act_info.json")) as f:
            info = json.load(f)
        sets = info["act_func_sets"]
        want = [s for s in sets if s["name"] == "natural_log_exp_and_others"]
        rest = [s for s in sets if s["name"] != "natural_log_exp_and_others"]
        info["act_func_sets"] = want + rest
        with open(os.path.join(tmp, "act_info.json"), "w") as f:
            json.dump(info, f)
        os.replace(tmp, dst)
    path = os.path.join(dst, "act_info.json")
    os.environ["BASS_ACT_ROOT_JSON_PATH"] = path

    def patched(module_arch):
        with open(path) as af:
            act_info = json.load(af)
        return {
            ent["name"]: {
                mybir.ActivationFunctionType.from_pwp(v) for v in ent["act"].keys()
            }
            for ent in act_info["act_func_sets"]
        }

    hw_specs.get_activation_tables = patched
    _bacc.get_activation_tables = patched
    from concourse import bass_interp as _bi

    _bi.get_activation_tables = patched


B, S, C = 8, 1024, 1024
H, DK, DV = 16, 64, 64
P = 128
NT = 8  # number of 128-tiles along S / C / H*DK
CH = 512  # projection free-dim chunk (one PSUM bank of fp32)
NCH = S // CH
QC = 512  # attention q-chunk
NQC = S // QC


FP = mybir.dt.float32
BF = mybir.dt.bfloat16
BF_NP = ml_dtypes.bfloat16
AFT = mybir.ActivationFunctionType
ALU = mybir.AluOpType


def build_nc() -> bass.Bass:
    _setup_act_tables()
    nc = bacc.Bacc()

    xq = nc.dram_tensor("xq", [C, S], BF, kind="ExternalInput")
    xk = nc.dram_tensor("xk", [C, S], BF, kind="ExternalInput")
    xv = nc.dram_tensor("xv", [C, S], BF, kind="ExternalInput")
    wq = nc.dram_tensor("wq", [C, H * DK], BF, kind="ExternalInput")
    wk = nc.dram_tensor("wk", [C, H * DK], BF, kind="ExternalInput")
    wv = nc.dram_tensor("wv", [C, H * DV], BF, kind="ExternalInput")
    wo = nc.dram_tensor("wo", [H * DV, C], BF, kind="ExternalInput")
    bqd = nc.dram_tensor("bq", [P, NT], FP, kind="ExternalInput")
    bkd = nc.dram_tensor("bk", [P, NT], FP, kind="ExternalInput")
    bvd = nc.dram_tensor("bv", [P, H * DV], BF, kind="ExternalInput")
    bod = nc.dram_tensor("bo", [1, C], BF, kind="ExternalInput")
    y = nc.dram_tensor("y", [S, C], FP, kind="ExternalOutput")

    # constants baked into the NEFF, one bf16 tensor (single DMA):
    # cols 0:640   = causal mask for a band-block pair stored as [512|384]:
    #                [tri | ones(384) | tri]  (tri[t,q] = 1 iff t<=q)
    # cols 640:1024 = mask for a pair stored as [256|128]: [tri | ones | tri]
    tri = np.triu(np.ones((P, P), np.float32))
    ones = np.ones((P, P), np.float32)
    pat2 = np.zeros((P, P), np.float32)
    pat2[0, 0:DV] = 1.0
    pat2[32, DV:P] = 1.0
    cpack_np = np.concatenate(
        [tri, ones, ones, ones, tri, tri, ones, tri, pat2], axis=1
    )
    cpack_d = nc.inline_tensor(cpack_np.astype(BF_NP), "cpack")

    xq_r = xq.rearrange("(ko p) s -> p ko s", p=P)
    xk_r = xk.rearrange("(ko p) s -> p ko s", p=P)
    xv_r = xv.rearrange("(ko p) s -> p ko s", p=P)
    wq_r = wq.rearrange("(ko p) m -> p ko m", p=P)
    wk_r = wk.rearrange("(ko p) m -> p ko m", p=P)
    wv_r = wv.rearrange("(ko p) m -> p ko m", p=P)
    wo_r = wo.rearrange("(ko p) c -> p ko c", p=P)
    y_r = y.rearrange("(mo p) c -> p mo c", p=P)

    with tile.TileContext(nc) as tc, ExitStack() as octx:
        const = octx.enter_context(tc.tile_pool(name="const", bufs=1))
        qk = octx.enter_context(tc.tile_pool(name="qk", bufs=1))
        opool = octx.enter_context(tc.tile_pool(name="oT", bufs=1))

        qT_sb = qk.tile([P, NT, S], BF, tag="qT")
        kT2_sb = qk.tile([P, H, S], BF, tag="kT2")
        v_sb = qk.tile([P, NT, H, DV + 1], BF, tag="v")
        oT_sb = opool.tile([P, NT, S], BF, tag="oT")

        # ---------------- pools + DMAs ----------------
        wpool = octx.enter_context(tc.tile_pool(name="wqkv", bufs=2))
        xpool = octx.enter_context(tc.tile_pool(name="xin", bufs=2))

        wq_sb = wpool.tile([P, NT, H * DK], BF, tag="w", name="wq_sb")
        xq_sb = xpool.tile([P, NT, S], BF, tag="x", name="xq_sb")
        wk_sb = wpool.tile([P, NT, H * DK], BF, tag="w", name="wk_sb")
        xk_sb = xpool.tile([P, NT, S], BF, tag="x", name="xk_sb")
        # Q inputs + small consts first so the PE starts ASAP
        nc.sync.dma_start(wq_sb[:, 0], wq_r[:, 0])
        nc.sync.dma_start(xq_sb[:, 0], xq_r[:, 0])
        bq_sb = const.tile([P, NT], FP, tag="bq")
        nc.sync.dma_start(bq_sb, bqd[:])
        bk_sb = const.tile([P, NT], FP, tag="bk")
        nc.sync.dma_start(bk_sb, bkd[:])
        cpack_sb = const.tile([P, 1152], BF, tag="cpack")
        nc.sync.dma_start(cpack_sb, cpack_d[:])
        for kc in range(1, NT):
            nc.sync.dma_start(wq_sb[:, kc], wq_r[:, kc])
            nc.sync.dma_start(xq_sb[:, kc], xq_r[:, kc])
        for kc in range(NT):
            nc.sync.dma_start(wk_sb[:, kc], wk_r[:, kc])
            nc.sync.dma_start(xk_sb[:, kc], xk_r[:, kc])

        bo_sb = const.tile([1, C], BF, tag="bo")
        nc.sync.dma_start(bo_sb, bod[:])
        borep_sb = const.tile([P, C], BF, tag="borep")
        nc.gpsimd.partition_broadcast(borep_sb, bo_sb)
        bv_sb = const.tile([P, H * DV], BF, tag="bv")
        nc.sync.dma_start(bv_sb, bvd[:])

        nc.vector.memset(v_sb[:, :, :, DV], 1.0)
        # zero the unused half of each head's K^T tile so St matmuls can
        # contract over the full 128 partitions (the zero lhsT rows
        # nullify the other head's Q rows)
        for h in range(H):
            hz = DK if h % 2 == 0 else 0
            nc.gpsimd.memset(kT2_sb[hz : hz + DK, h, :], 0.0)

        # V inputs reuse Q's buffers (free once Q's matmuls finish); the
        # output-projection weight reuses K's buffer after K's last matmul
        wv_sb = wpool.tile([P, NT, H * DV], BF, tag="w", name="wv_sb")
        xv_sb = xpool.tile([P, NT, S], BF, tag="x", name="xv_sb")
        for kc in range(NT):
            nc.sync.dma_start(wv_sb[:, kc], wv_r[:, kc])
            nc.sync.dma_start(xv_sb[:, kc], xv_r[:, kc])
        wo_sb = wpool.tile([P, NT, C], BF, tag="w", name="wo_sb")
        nc.sync.dma_start(wo_sb, wo_r)

        # ---------------- Q + K(n=0) projections ----------------
        # out[hk, s]; lhsT = w tile [c, hk], rhs = x^T [c, s]; kc-outer so
        # matmuls chase the input DMAs; evacuation (+bias) on the idle ACT.
        def qk_evac_m(ps, m, n, b_sb, out_sb):
            sl = slice(n * CH, (n + 1) * CH)
            if out_sb is not None:
                nc.vector.tensor_scalar_add(
                    out_sb[:, m, sl], ps, b_sb[:, m : m + 1]
                )
            else:  # K^T: split the head pair into per-head tiles
                nc.vector.tensor_scalar_add(
                    kT2_sb[0:DK, 2 * m, sl], ps[0:DK], b_sb[0:DK, m : m + 1]
                )
                nc.vector.tensor_scalar_add(
                    kT2_sb[DK:P, 2 * m + 1, sl], ps[DK:P], b_sb[DK:P, m : m + 1]
                )

        with ExitStack() as ictx:
            psproj = ictx.enter_context(
                tc.tile_pool(name="psproj", bufs=8, space="PSUM")
            )
            # Q n=0: kc-outer so matmuls chase the input DMAs
            psums = []
            for kc in range(NT):
                for m in range(NT):
                    if kc == 0:
                        psums.append(psproj.tile(
                            [P, CH], FP, tag="proj", name=f"proj_ps_{m}"))
                    nc.tensor.matmul(
                        psums[m],
                        wq_sb[:, kc, m * P : (m + 1) * P],
                        xq_sb[:, kc, 0:CH],
                        start=(kc == 0), stop=(kc == NT - 1),
                    )
            for m in range(NT):
                qk_evac_m(psums[m], m, 0, bq_sb, qT_sb)
            # Q n=1 / K n=0: m-outer so each tile's evacuation overlaps the
            # next tile's matmuls (keeps the DVE queue clear of bursts)
            for w_sb, x_sb, b_sb, out_sb, n in (
                (wq_sb, xq_sb, bq_sb, qT_sb, 1),
                (wk_sb, xk_sb, bk_sb, None, 0),
            ):
                for m in range(NT):
                    ps = psproj.tile([P, CH], FP, tag="proj", name=f"proj_ps_{m}")
                    for kc in range(NT):
                        nc.tensor.matmul(
                            ps,
                            w_sb[:, kc, m * P : (m + 1) * P],
                            x_sb[:, kc, n * CH : (n + 1) * CH],
                            start=(kc == 0), stop=(kc == NT - 1),
                        )
                    qk_evac_m(ps, m, n, b_sb, out_sb)

        # remaining PSUM pools: 3 (mix) + 3 (scores) + 2 (P@V out) = 8 banks
        ps_mix = octx.enter_context(tc.tile_pool(name="ps_mix", bufs=2, space="PSUM"))
        ps_st = octx.enter_context(tc.tile_pool(name="ps_st", bufs=2, space="PSUM"))
        ps_o = octx.enter_context(tc.tile_pool(name="ps_o", bufs=2, space="PSUM"))
        ppool = octx.enter_context(tc.tile_pool(name="p", bufs=16))
        rpool = octx.enter_context(tc.tile_pool(name="r", bufs=3))
        oupool = octx.enter_context(tc.tile_pool(name="ou", bufs=4))
        ypool = octx.enter_context(tc.tile_pool(name="y", bufs=2))

        def kn1_piece(m):
            """K projection n=1 for one m-tile (filler between ch0 stages)."""
            ps = ps_mix.tile([P, CH], FP, tag="mix", name=f"kn1_{m}")
            for kc in range(NT):
                nc.tensor.matmul(
                    ps, wk_sb[:, kc, m * P : (m + 1) * P],
                    xk_sb[:, kc, CH : 2 * CH],
                    start=(kc == 0), stop=(kc == NT - 1),
                )
            sl = slice(CH, 2 * CH)
            nc.scalar.activation(
                kT2_sb[0:DK, 2 * m, sl], ps[0:DK], AFT.Identity,
                bias=bk_sb[0:DK, m : m + 1],
            )
            nc.scalar.activation(
                kT2_sb[DK:P, 2 * m + 1, sl], ps[DK:P], AFT.Identity,
                bias=bk_sb[DK:P, m : m + 1],
            )

        def vm_piece(m):
            """V projection for key block m: out[s, hv]; lhsT = x^T tile."""
            pss = []
            for nh in range(NCH):
                ps = ps_mix.tile([P, CH], FP, tag="mix", name=f"v_{m}_{nh}")
                pss.append(ps)
            for kc in range(NT):
                for nh in range(NCH):
                    nc.tensor.matmul(
                        pss[nh], xv_sb[:, kc, m * P : (m + 1) * P],
                        wv_sb[:, kc, nh * CH : (nh + 1) * CH],
                        start=(kc == 0), stop=(kc == NT - 1),
                    )
            for nh in range(NCH):
                nc.vector.tensor_tensor(
                    v_sb[:, m, 8 * nh : 8 * (nh + 1), 0:DV],
                    pss[nh].rearrange("p (h v) -> p h v", v=DV),
                    bv_sb[:, nh * CH : (nh + 1) * CH].rearrange(
                        "p (h v) -> p h v", v=DV
                    ),
                    ALU.add,
                )

        # ---------------- attention + interleaved projections ------------
        # Chunk 0 (keys 0:512) needs only K n=0 and V blocks 0..3, so its
        # ACT-bound scores+exp interleave with the K n=1 matmuls as PE
        # filler; chunk 1 software-pipelines with chunk-0 output-projection
        # pieces. Filler is woven BETWEEN score groups so the in-order PE
        # never stalls on the score-PSUM ring while work waits behind it.
        state = {}

        def blocks_of(j):
            """(i, qoff, width, group, loc): key blocks for q-chunk j packed
            pairwise into score-PSUM groups. The last 4 blocks form the
            causal diagonal band; block 4j+k only covers q-cols 128k:512."""
            out = []
            for i in range(4 * j + 4):
                qoff = max(0, (i - 4 * j) * 128)
                g, first = divmod(i, 2)
                loc = 0 if first == 0 else out[-1][2]
                out.append((i, qoff, QC - qoff, g, loc))
            return out

        def st_sub(j, hp, sub, g):
            """Scores + exp + causal mask for one pair-group of one head."""
            qlo = j * QC
            st = state.setdefault((j, hp), {"pchs": {}})
            gb = [b for b in blocks_of(j) if b[3] == g]
            h = 2 * hp + sub
            stp = ps_st.tile([P, 1024], FP, tag="st", name=f"st_{j}_{hp}_{sub}_{g}")
            for i, qoff, w, _, loc in gb:
                nc.tensor.matmul(
                    stp[:, loc : loc + w],
                    kT2_sb[:, h, i * P : (i + 1) * P],
                    qT_sb[:, hp, qlo + qoff : qlo + QC],
                    start=True,
                    stop=True,
                )
            span = gb[-1][4] + gb[-1][2]
            pch = ppool.tile([P, 1024], BF, tag="p", name=f"p_{j}_{hp}_{sub}_{g}")
            nc.scalar.activation(pch[:, 0:span], stp[:, 0:span], AFT.Exp)
            if gb[0][0] >= 4 * j:  # band pair: combined triangular mask
                # the last head pairs mask on the idle Pool engine so the
                # busy DVE queue is off the tail's critical path
                eng = nc.gpsimd if (j == 1 and hp >= 6) else nc.vector
                if gb[0][2] == QC:  # [512|384] pair
                    eng.tensor_tensor(
                        pch[:, 0:640], pch[:, 0:640], cpack_sb[:, 0:640], ALU.mult
                    )
                else:  # [256|128] pair
                    eng.tensor_tensor(
                        pch[:, 0:384], pch[:, 0:384], cpack_sb[:, 640:1024], ALU.mult
                    )
            st["pchs"][(sub, g)] = pch

        def pv_sub(j, hp, sub):
            st = state[(j, hp)]

            h = 2 * hp + sub
            blks = blocks_of(j)
            pos = ps_o.tile([P, QC], FP, tag="o", name=f"po_{j}_{hp}_{sub}")
            for i, qoff, w, g, loc in blks:
                nc.tensor.matmul(
                    pos[0 : DV + 1, qoff:QC],
                    v_sb[:, i, h, :],
                    st["pchs"][(sub, g)][:, loc : loc + w],
                    start=(i == 0),
                    stop=(i == len(blks) - 1),
                )
            # immediate PSUM->SBUF evacuation so the bank recycles fast;
            # the last head pairs evacuate on ACT (idle once exps are done)
            ou = oupool.tile([DV, QC], FP, tag="ou", name=f"ou_{j}_{hp}_{sub}")
            r1 = rpool.tile([1, QC], BF, tag="r1", name=f"r1_{j}_{hp}_{sub}")
            if j == 1 and hp >= 6:
                nc.scalar.copy(ou, pos[0:DV])
                nc.scalar.copy(r1, pos[DV : DV + 1])
            else:
                nc.vector.tensor_copy(out=ou, in_=pos[0:DV])
                nc.vector.tensor_copy(out=r1, in_=pos[DV : DV + 1])
            st[("r1", sub)] = r1
            st[("ou", sub)] = ou

        def norm_stage(j, hp):
            """oT = ou / r: one K=2 matmul replicates both subs' denominator
            rows across the partition halves; fast reciprocal + mults on DVE."""
            st = state.pop((j, hp))
            qlo = j * QC
            for sub in (0, 1):
                hm = sub * DV
                rrep = ps_mix.tile(
                    [DV, CH], FP, tag="mix", name=f"rrep_{j}_{hp}_{sub}"
                )
                nc.tensor.matmul(
                    rrep[:, 0:QC], cpack_sb[0:1, P : P + DV], st[("r1", sub)],
                    start=True, stop=True,
                )
                rrinv = rpool.tile(
                    [DV, QC], FP, tag="rrinv", name=f"rrinv_{j}_{hp}_{sub}"
                )
                nc.vector.reciprocal_approx_fast(rrinv, rrep[:, 0:QC])
                nc.vector.tensor_tensor(
                    oT_sb[hm : hm + DV, hp, qlo : qlo + QC],
                    st[("ou", sub)],
                    rrinv,
                    ALU.mult,
                )

        def outproj_piece(m, n, pool=None):
            if pool is None:
                py = ps_mix.tile([P, CH], FP, tag="mix", name=f"py_{m}_{n}")
            else:
                py = pool.tile([P, 1024], FP, tag="st", name=f"py_{m}_{n}")[:, 0:CH]
            for kc in range(NT):
                nc.tensor.matmul(
                    py,
                    oT_sb[:, kc, m * P : (m + 1) * P],
                    wo_sb[:, kc, n * CH : (n + 1) * CH],
                    start=(kc == 0),
                    stop=(kc == NT - 1),
                )
            yt = ypool.tile([P, CH], FP, tag="y")
            nc.vector.tensor_tensor(
                yt, py, borep_sb[:, n * CH : (n + 1) * CH], ALU.add
            )
            nc.sync.dma_start(y_r[:, m, n * CH : (n + 1) * CH], yt)

        # chunk 0 with K n=1 and V-projection pieces woven between score
        # groups (V's input DMAs land under the early steps' compute)
        fillers = [
            ("kn1", 0), ("vm", 0), ("vm", 1), ("kn1", 1), ("vm", 2),
            ("vm", 3), ("kn1", 2), ("kn1", 3), ("kn1", 4), ("kn1", 5),
            ("kn1", 6), ("kn1", 7), ("vm", 4), ("vm", 5), ("vm", 6),
            ("vm", 7),
        ]

        def filler():
            if fillers:
                kind, m = fillers.pop(0)
                (kn1_piece if kind == "kn1" else vm_piece)(m)

        for hp in range(H // 2):
            st_sub(0, hp, 0, 0)
            st_sub(0, hp, 0, 1)
            filler()
            if hp >= 3:
                pv_sub(0, hp - 3, 0)
            st_sub(0, hp, 1, 0)
            st_sub(0, hp, 1, 1)
            filler()
            if hp >= 3:
                pv_sub(0, hp - 3, 1)
            if hp >= 4:
                norm_stage(0, hp - 4)
        # drain chunk 0 (remaining V pieces interleave as filler)
        for hp in (5, 6, 7):
            for sub in (0, 1):
                pv_sub(0, hp, sub)
            filler()
            norm_stage(0, hp - 1)
        filler()
        norm_stage(0, 7)

        op_queue = [(m, n) for m in range(4) for n in range(NCH)]

        # chunk 1 software-pipelined with chunk-0 output-projection filler
        for hp in range(H // 2):
            st_sub(1, hp, 0, 0)
            st_sub(1, hp, 0, 1)
            if hp >= 1:
                pv_sub(1, hp - 1, 0)
            st_sub(1, hp, 0, 2)
            st_sub(1, hp, 0, 3)
            if hp >= 1:
                pv_sub(1, hp - 1, 1)
            st_sub(1, hp, 1, 0)
            st_sub(1, hp, 1, 1)
            outproj_piece(*op_queue.pop(0))
            st_sub(1, hp, 1, 2)
            st_sub(1, hp, 1, 3)
            if hp >= 2:
                norm_stage(1, hp - 2)
        for sub in (0, 1):
            pv_sub(1, 7, sub)
        norm_stage(1, 6)
        norm_stage(1, 7)
        op_queue += [(m, n) for m in range(4, NT) for n in range(NCH)]
        alt = 0
        while op_queue:
            outproj_piece(*op_queue.pop(0), pool=(ps_st if alt % 2 else None))
            alt += 1

    nc.finalize()
    return nc


_NC_CACHE = None


def _get_nc() -> bass.Bass:
    global _NC_CACHE
    if _NC_CACHE is None:
        _NC_CACHE = build_nc()
    return _NC_CACHE


def prep_shared(Wq, bq, Wk, bk, Wv, bv, Wo, bo):
    """Host-side packing of weights/biases (shared by all cores)."""
    scale = 1.0 / math.sqrt(DK)
    Wq = np.asarray(Wq, np.float32)
    Wk = np.asarray(Wk, np.float32)
    Wv = np.asarray(Wv, np.float32)
    Wo = np.asarray(Wo, np.float32)
    out = {
        "wq": np.ascontiguousarray(
            (Wq.transpose(1, 0, 2).reshape(C, H * DK) * scale).astype(BF_NP)
        ),
        "wk": np.ascontiguousarray(
            Wk.transpose(1, 0, 2).reshape(C, H * DK).astype(BF_NP)
        ),
        "wv": np.ascontiguousarray(
            Wv.transpose(1, 0, 2).reshape(C, H * DV).astype(BF_NP)
        ),
        "wo": Wo.astype(BF_NP),
        "bq": np.ascontiguousarray(
            (np.asarray(bq, np.float32).reshape(H * DK) * scale)
            .reshape(NT, P)
            .T.astype(np.float32)
        ),
        "bk": np.ascontiguousarray(
            np.asarray(bk, np.float32).reshape(NT, P).T.astype(np.float32)
        ),
        "bv": np.ascontiguousarray(
            np.broadcast_to(
                np.asarray(bv, np.float32).reshape(1, H * DV), (P, H * DV)
            ).astype(BF_NP)
        ),
        "bo": np.ascontiguousarray(
            np.asarray(bo, np.float32).reshape(1, C).astype(BF_NP)
        ),
    }
    return out


def prep_core(q_embs_b, k_embs_b, v_embs_b):
    return {
        "xq": np.ascontiguousarray(np.asarray(q_embs_b, np.float32).T.astype(BF_NP)),
        "xk": np.ascontiguousarray(np.asarray(k_embs_b, np.float32).T.astype(BF_NP)),
        "xv": np.ascontiguousarray(np.asarray(v_embs_b, np.float32).T.astype(BF_NP)),
    }


def kernel(q_embs, k_embs, v_embs, Wq, bq, Wk, bk, Wv, bv, Wo, bo, **run_kwargs):
    nc = _get_nc()
    shared = prep_shared(Wq, bq, Wk, bk, Wv, bv, Wo, bo)
    q_embs = np.asarray(q_embs, np.float32)
    k_embs = np.asarray(k_embs, np.float32)
    v_embs = np.asarray(v_embs, np.float32)
    in_maps = []
    for b in range(B):
        m = dict(shared)
        m.update(prep_core(q_embs[b], k_embs[b], v_embs[b]))
        in_maps.append(m)
    res = run_bass_kernel_spmd(nc, in_maps, core_ids=list(range(B)), **run_kwargs)
    out = np.stack([res.results[i]["y"] for i in range(B)], axis=0)
    if run_kwargs:
        kernel.last_results = res
    return out


if __name__ == "__main__":
    rng = np.random.default_rng(0)
    inputs = {
        "q_embs": rng.standard_normal((B, S, C), np.float32),
        "k_embs": rng.standard_normal((B, S, C), np.float32),
        "v_embs": rng.standard_normal((B, S, C), np.float32),
        "Wq": rng.standard_normal((H, C, DK), np.float32) * 0.02,
        "bq": np.zeros((H, DK), np.float32),
        "Wk": rng.standard_normal((H, C, DK), np.float32) * 0.02,
        "bk": np.zeros((H, DK), np.float32),
        "Wv": rng.standard_normal((H, C, DV), np.float32) * 0.02,
        "bv": np.zeros((H, DV), np.float32),
        "Wo": rng.standard_normal((H * DV, C), np.float32) * 0.02,
        "bo": np.zeros((C,), np.float32),
    }
    out = kernel(**inputs)
    print(out.shape, out.dtype)
